# revision 64
# baseline (speedup 1.0000x reference)
"""MelSpectrogram + PCEN Trainium2 kernel v7 (8-core data parallel).

Pipeline per core (8 batch elements):
  host: reflect-pad, hop-block transpose (512 x 2528), fp16 cast
  DVE:  u+/- = x_t +/- x_t+2; v0/v4 = u+[r'] +/- u+[r'+256]
        (radix folds: A-step and the mod-8 half-contraction)
  PE:   hop-block DFT via matmul -> A tiles (packed ft-major E, fp16,
        1/16-scaled); mod-8 planes p0/p4 contract only 256 rows against
        v0/v4; f=1024 folded into the f=0 row of the p0 tile
  ACT:  PSUM->SBUF evac (f32->f16), width-scaled
  DVE:  X-step as flat per-plane 2-dim ops (multi-dim APs and any
        concurrent gpsimd SBUF traffic both degrade DVE throughput)
  DVE:  h = 0.5 x; wconv tmp/xw as flat per-comp subs on full chunks
  PE:   q+-1 boundary shift tiles via 2 shift-matrix matmuls per comp
  ACT:  square (in-place into tmp)
  PE:   mel projection (fb folded with comp-duplication + s + scale)
  PE:   PCEN IIR smoother as Toeplitz matmuls over DMA-transposed mel
        (b0..b6); the last b uses a chained tensor_tensor_scan split
        across the final two chunks
  ACT/DVE: PCEN pointwise ln/exp chain, batched over pairs of b

Mod-8 plane-major f-slot layout per comp c (r=0: cos, i=1: -sin),
8 tiles of 128 per comp: tile p holds f = 8q+p, q=0..127; tile 0 row 0
holds f=1024. comp i tiles are offset by 8 tiles.
"""

import math
from contextlib import ExitStack

import numpy as np

SR, N_FFT, HOP, N_MELS = 32000, 2048, 512, 128
F_MIN, F_MAX = 20.0, 16000.0
EPS, S, ALPHA, DELTA, R = 1e-6, 0.025, 0.98, 2.0, 0.5
NBINS = N_FFT // 2 + 1
T = 313
SBLK = 316
PAD = N_FFT // 2
B_TOTAL, L_WAVE = 64, 160000
N_CORES = 8

SC = 16.0    # E scale (E = E_true/SC)
SCM = 16.0   # mel scale
SCE = 256.0  # e2 scale (keeps (eps+m)^-alpha comfortably in range)
SCL = 8.0    # LT scale (keeps fp16 LT entries in normal range)
W = 512
W16 = 16 * W
W8 = 8 * W

# PCEN Toeplitz tiling: three overlapping 128-frame transpose tiles;
# each LT tile only "owns" the tau rows in its responsibility range.
LT_TILES = [(0, 0, 128), (128, 128, 256), (185, 256, 313)]  # (t0, lo, hi)


def _slot_of(f, c):
    # mod-8 planes; f=1024 folded into the (unused) f=0 slot of plane 0
    if f == 1024:
        return c * 1024
    p, q = f % 8, f // 8
    return c * 1024 + p * 128 + q


def _mel_fbank():
    def hz2mel(f):
        return 2595.0 * np.log10(1.0 + np.asarray(f, np.float64) / 700.0)

    def mel2hz(m):
        return 700.0 * (10.0 ** (np.asarray(m, np.float64) / 2595.0) - 1.0)

    all_freqs = np.linspace(0.0, SR / 2.0, NBINS)
    m_pts = np.linspace(hz2mel(F_MIN), hz2mel(F_MAX), N_MELS + 2)
    f_pts = mel2hz(m_pts)
    f_diff = np.diff(f_pts)
    slopes = f_pts[None, :] - all_freqs[:, None]
    down = -slopes[:, :-2] / f_diff[:-1]
    up = slopes[:, 2:] / f_diff[1:]
    return np.maximum(0.0, np.minimum(down, up))


def _build_consts():
    r = np.arange(HOP)
    rp = np.arange(256)
    E = np.zeros((HOP, 2048), np.float64)
    for f in range(1, NBINS):  # f=0 dropped; f=1024 takes its slot
        th = 2.0 * np.pi * f * r / N_FFT
        if f % 8 in (0, 4):
            # p0/p4 contract only r' = 0..255 against v0/v4
            thp = 2.0 * np.pi * f * rp / N_FFT
            E[0:256, _slot_of(f, 0)] = np.cos(thp) / SC
            E[0:256, _slot_of(f, 1)] = -np.sin(thp) / SC
        else:
            E[:, _slot_of(f, 0)] = np.cos(th) / SC
            E[:, _slot_of(f, 1)] = -np.sin(th) / SC
    fb = _mel_fbank()
    # the slot-fold relies on fb rows 0/1/1024 being empty
    assert abs(fb[1024]).max() < 1e-9
    assert abs(fb[0]).max() < 1e-9 and abs(fb[1]).max() < 1e-9
    fb2 = np.zeros((2048, N_MELS), np.float64)
    for f in range(1024):
        wgt = fb[f] * (SC * SC / 4.0) * S / SCM
        for c in range(2):
            fb2[_slot_of(f, c)] = wgt
    # boundary-shift matrices (tmp[p0] -= 0.5 x[p7,q-1]; xw[p7] -= 0.5 x[p0,q+1])
    sdn = 0.5 * np.eye(128, k=1)
    supl = 0.5 * np.eye(128, k=-1)
    supl[0, 127] = 0.5  # f=1023's +1 neighbor is f=1024 = p0 row 0
    sh = np.concatenate([sdn, supl], axis=1)
    # LT[j][tau_local, t] = (1-S)^(t - tau) * SCL for tau in the tile's
    # responsibility range [lo, hi) and tau <= t (s itself is folded into
    # fb2, so melb = s*mel/SCM and msp = SCL*m/SCM).
    t = np.arange(T)
    lt = np.zeros((3, 128, T), np.float64)
    for j, (t0, lo, hi) in enumerate(LT_TILES):
        for tau in range(lo, hi):
            msk = t >= tau
            lt[j, tau - t0, msk] = (1.0 - S) ** (t[msk] - tau) * SCL
    return E, fb2, sh, lt


# Full-width chunks, then a geometrically-decreasing tail (elementwise is
# width-scaled, and the final serial drain scales with the LAST chunk).
def _make_chunks(NC):
    chunks = []
    co = 0
    while co < NC - 3:
        rem = NC - co
        if rem <= 96:
            chunks.append((co, rem)); co += rem - 3
        elif rem <= W:
            h = max(96, (rem * 3 + 4) // 5)
            chunks.append((co, h)); co += h - 3
        else:
            chunks.append((co, W)); co += W - 3
    return chunks


def _split_multiwaits(nc, limit=1):
    """This walrus build accepts at most `limit` sync-waits per instruction
    (and none at all on scalar_tensor_tensor); move excess waits onto
    preceding same-engine NoOps."""
    import bass_rust
    import concourse.mybir as mybir

    for fn in nc.m.functions:
        for b in fn.blocks:
            insts = b.instructions
            new = []
            changed = False
            for i in insts:
                lim = limit
                if (
                    isinstance(i, mybir.InstTensorScalarPtr)
                    and getattr(i, "is_scalar_tensor_tensor", False)
                    and not getattr(i, "is_tensor_tensor_scan", False)
                ):
                    lim = 0
                si = i.sync_info
                if si is not None and len(si.on_wait) > lim:
                    waits = list(si.on_wait)
                    keep = waits[len(waits) - lim :] if lim else []
                    rest = waits[: len(waits) - lim] if lim else waits
                    step = max(limit, 1)
                    for k in range(0, len(rest), step):
                        chunk = rest[k : k + step]
                        nop = mybir.InstNoOp(
                            name=f"{i.name}-wsplit{k}", ins=[], outs=[]
                        )
                        nop.engine = i.engine
                        nop.sync_info = bass_rust.SyncInfo(
                            on_wait=chunk, on_update=[]
                        )
                        new.append(nop)
                        changed = True
                    si.on_wait = keep
                new.append(i)
            if changed:
                b.instructions = new


def build_nc(BL=8, split=True):
    import concourse.bass as bass
    import concourse.mybir as mybir
    from concourse import tile

    f16 = mybir.dt.float16
    f32 = mybir.dt.float32
    ALU = mybir.AluOpType
    ACTF = mybir.ActivationFunctionType

    NC = BL * SBLK
    chunks = _make_chunks(NC)
    ncb = len(chunks)

    # packed ft-major E: per comp, planes 0..7 with 2 rc-blocks for p0/p4
    # (contraction 256 against v) and 4 for the rest -> 3584 cols per comp
    plane_nrc = [2, 4, 4, 4, 2, 4, 4, 4]
    ft_base = []
    off = 0
    for c in range(2):
        for p in range(8):
            ft_base.append(off)
            off += plane_nrc[p] * 128
    E_COLS = off  # 7168

    nc = bass.Bass("TRN2", target_bir_lowering=False, debug=False)
    xt_d = nc.dram_tensor("xt", [4, 128, NC], f16, kind="ExternalInput")
    e_d = nc.dram_tensor("e", [128, E_COLS], f16, kind="ExternalInput")
    fb_d = nc.dram_tensor("fb", [128, 2048], f16, kind="ExternalInput")
    sh_d = nc.dram_tensor("sh", [128, 256], f16, kind="ExternalInput")
    lt_d = nc.dram_tensor("lt", [3, 128, T], f16, kind="ExternalInput")
    y_d = nc.dram_tensor("y", [BL, 128, T], f32, kind="ExternalOutput")

    with tile.TileContext(nc) as tc, ExitStack() as top:
        cpool = top.enter_context(tc.tile_pool(name="consts", bufs=1))
        xb = cpool.tile([128, 4 * NC], f16)
        eb = cpool.tile([128, E_COLS], f16)
        fbb = cpool.tile([128, 2048], f16)
        melb = cpool.tile([128, NC], f16)
        shb = cpool.tile([128, 256], f16)
        ltb = cpool.tile([128, 3 * T], f16)
        decf = cpool.tile([128, T], f16)
        ltv = ltb[:, :].rearrange("p (k t) -> p k t", k=3)

        xbv = xb[:, :].rearrange("p (rc c) -> p rc c", rc=4)

        shv = shb[:, :].rearrange("p (k c) -> p k c", k=2)
        # head slices on parallel DGE queues so dft(0) can begin early;
        # everything else is issued after the first chunk's emission
        EH, XH = 768, min(516, NC)
        x_t = xt_d.ap().rearrange("rc p c -> p rc c")
        nc.sync.dma_start(xbv[:, :, 0:XH], x_t[:, :, 0:XH])
        nc.scalar.dma_start(eb[:, 0:256], e_d.ap()[:, 0:256])
        nc.scalar.dma_start(eb[:, 256:EH], e_d.ap()[:, 256:EH])
        nc.gpsimd.dma_start(eb[:, EH : E_COLS // 2], e_d.ap()[:, EH : E_COLS // 2])
        nc.sync.dma_start(
            eb[:, E_COLS // 2 : E_COLS], e_d.ap()[:, E_COLS // 2 : E_COLS]
        )
        bias_t = cpool.tile([128, 4], f32)

        def emit_late_loads():
            nc.scalar.dma_start(shb[:, :], sh_d.ap()[:, :])
            nc.sync.dma_start(fbb[:, :], fb_d.ap()[:, :])
            if XH < NC:
                nc.scalar.dma_start(xbv[:, :, XH:NC], x_t[:, :, XH:NC])
            for k in range(3):
                nc.sync.dma_start(ltv[:, k, :], lt_d.ap()[k])
            nc.gpsimd.memset(decf[:, :], 1.0 - S)
            nc.gpsimd.memset(bias_t[:, 0:1], EPS)
            nc.gpsimd.memset(bias_t[:, 1:2], math.log(SCE))
            nc.gpsimd.memset(bias_t[:, 2:3], DELTA)
            nc.gpsimd.memset(bias_t[:, 3:4], 0.0)

        with ExitStack() as cph:
            yps = cph.enter_context(tc.tile_pool(name="yps", bufs=2, space="PSUM"))
            xsps = cph.enter_context(tc.tile_pool(name="xsps", bufs=3, space="PSUM"))
            mps = cph.enter_context(tc.tile_pool(name="mps", bufs=1, space="PSUM"))
            p_u = cph.enter_context(tc.tile_pool(name="p_u", bufs=2))
            p_v = cph.enter_context(tc.tile_pool(name="p_v", bufs=2))
            p_a = cph.enter_context(tc.tile_pool(name="p_a", bufs=3))
            p_x = cph.enter_context(tc.tile_pool(name="p_x", bufs=2))
            p_h = cph.enter_context(tc.tile_pool(name="p_h", bufs=1))
            p_t = cph.enter_context(tc.tile_pool(name="p_t", bufs=2))
            p_ec = cph.enter_context(tc.tile_pool(name="p_ec", bufs=3))
            p_sc = cph.enter_context(tc.tile_pool(name="p_sc", bufs=1))
            p_out = cph.enter_context(tc.tile_pool(name="p_out", bufs=1))

            us = [None] * ncb
            vs = [None] * ncb
            asbs = [None] * ncb

            def emit_u(cj):
                co, w = chunks[cj]
                u = p_u.tile([128, 2, 4, W], f16, tag="u")
                n2 = min(w, NC - co - 2)
                if n2 < w:
                    nc.gpsimd.memset(u[:, :, :, n2:w], 0.0)
                nc.vector.tensor_add(
                    u[:, 0, :, 0:n2], xbv[:, :, co : co + n2],
                    xbv[:, :, co + 2 : co + 2 + n2],
                )
                nc.vector.tensor_sub(
                    u[:, 1, :, 0:n2], xbv[:, :, co : co + n2],
                    xbv[:, :, co + 2 : co + 2 + n2],
                )
                v = p_v.tile([128, 2, 2, W], f16, tag="v")
                nc.vector.tensor_add(
                    v[:, 0, :, 0:w], u[:, 0, 0:2, 0:w], u[:, 0, 2:4, 0:w]
                )
                nc.vector.tensor_sub(
                    v[:, 1, :, 0:w], u[:, 0, 0:2, 0:w], u[:, 0, 2:4, 0:w]
                )
                us[cj] = u
                vs[cj] = v

            def emit_dft(cj, mid_cb=None, fps=range(8)):
                co, w = chunks[cj]
                u, v = us[cj], vs[cj]
                if asbs[cj] is None:
                    asb = p_a.tile([128, W16 + 8], f16, tag="asb")
                    asbs[cj] = asb
                    # The X-step's +1-shifted reads touch exactly column w
                    # of each of the 16 tiles (for w=W only the first pad
                    # byte). Zero them so buffer reuse never reads stale
                    # bytes.
                    if w == W:
                        nc.gpsimd.memset(asb[:, W16 : W16 + 1], 0.0)
                    else:
                        nc.gpsimd.memset(
                            asb[:, 0:W16].rearrange("p (t w) -> p t w", t=16)[
                                :, :, w : w + 1
                            ],
                            0.0,
                        )
                else:
                    asb = asbs[cj]
                for fp in fps:
                    if fp in (4, 7) and mid_cb is not None:
                        mid_cb(fp)
                    yp = yps.tile([128, 2 * W], f32, tag="yp")
                    for half in range(2):
                        ft = 2 * fp + half
                        pl = ft % 8
                        dst = yp[:, half * W : half * W + w]
                        eb0 = ft_base[ft]

                        def esl(rc):
                            return eb[:, eb0 + rc * 128 : eb0 + (rc + 1) * 128]

                        if pl in (0, 4):
                            vv = v[:, 0 if pl == 0 else 1]
                            for rc in range(2):
                                nc.tensor.matmul(
                                    dst, esl(rc), vv[:, rc, 0:w],
                                    start=(rc == 0), stop=(rc == 1),
                                )
                        else:
                            usel = 0 if pl in (2, 6) else 1
                            for rc in range(4):
                                nc.tensor.matmul(
                                    dst, esl(rc), u[:, usel, rc, 0:w],
                                    start=(rc == 0), stop=(rc == 3),
                                )
                    dst = asb[:, 2 * fp * W : (2 * fp + 2) * W].rearrange(
                        "p (t c) -> p t c", t=2
                    )[:, :, 0:w]
                    src = yp[:, :].rearrange("p (t c) -> p t c", t=2)[:, :, 0:w]
                    nc.scalar.copy(dst, src)
                asbs[cj] = asb

            def emit_X(ci, even_on_dve=False):
                co, w = chunks[ci]
                asb = asbs[ci]
                x = p_x.tile([128, W16], f16, tag="x")

                def pair(c, planes, off=0, src=None):
                    # [p, 2, 1, w] view of planes (b, b+4) of comp c
                    b = planes[0]
                    t0 = src if src is not None else asb
                    lo = c * W8 + off
                    return t0[:, lo : lo + W8].rearrange(
                        "p (a b w) -> p a b w", a=2, b=4
                    )[:, :, b : b + 1, 0:w]

                def one(c, p, off=0, src=None):
                    # flat [p, w] view of a single plane (2-dim keeps DVE 2x)
                    t0 = src if src is not None else asb
                    lo = c * W8 + p * W + off
                    return t0[:, lo : lo + w]

                # all on DVE: gpsimd shares the DVE SBUF port, and any
                # concurrent gpsimd traffic slows DVE ops ~1.8x (HW-measured)
                # even planes: aligned segmented views keep 2x; the +1-shifted
                # operand views are handled per-plane flat below
                for c in range(2):
                    for p in (0, 4):
                        nc.vector.tensor_add(
                            one(c, p, 0, x), one(c, p), one(c, p, 1)
                        )
                    for p in (2, 6):
                        nc.vector.tensor_sub(
                            one(c, p, 0, x), one(c, p), one(c, p, 1)
                        )
                # odd planes: flat per-plane DVE ops (multi-dim APs drop DVE
                # to 1x on HW, flat 2-dim stays 2x)
                for p in (1, 5):
                    # Xr = Ar + Ai[t+1]; Xi = Ai - Ar[t+1]
                    nc.vector.tensor_add(one(0, p, 0, x), one(0, p), one(1, p, 1))
                    nc.vector.tensor_sub(one(1, p, 0, x), one(1, p), one(0, p, 1))
                for p in (3, 7):
                    # Xr = Ar - Ai[t+1]; Xi = Ai + Ar[t+1]
                    nc.vector.tensor_sub(one(0, p, 0, x), one(0, p), one(1, p, 1))
                    nc.vector.tensor_add(one(1, p, 0, x), one(1, p), one(0, p, 1))
                return x

            def emit_xs(ci, x):
                co, w = chunks[ci]
                xv = x[:, :].rearrange("p (c t) -> p c t", c=2)
                t3s, t0s = [], []
                for c in range(2):
                    t3 = xsps.tile([128, W], f32, tag="xs")
                    nc.tensor.matmul(t3[:, 0:w], shv[:, 0, :],
                                     xv[:, c, 7 * W : 7 * W + w],
                                     start=True, stop=True)
                    t0 = xsps.tile([128, W], f32, tag="xs")
                    nc.tensor.matmul(t0[:, 0:w], shv[:, 1, :],
                                     xv[:, c, 0:w],
                                     start=True, stop=True)
                    t3s.append(t3)
                    t0s.append(t0)
                return (t3s, t0s)

            def emit_wconv(ci, x, xs):
                co, w = chunks[ci]
                full = w == W
                xv = x[:, :].rearrange("p (c t) -> p c t", c=2)
                xv4 = xv.rearrange("p c (t w) -> p c t w", t=8)
                h = p_h.tile([128, W16], f16, tag="h")
                hv = h[:, :].rearrange("p (c t) -> p c t", c=2)
                hv4 = hv.rearrange("p c (t w) -> p c t w", t=8)
                nc.vector.tensor_scalar_mul(
                    hv4[:, :, :, 0:w], xv4[:, :, :, 0:w], 0.5
                )
                tmp = p_t.tile([128, W16], f16, tag="tmp")
                tv = tmp[:, :].rearrange("p (c t) -> p c t", c=2)
                tv4 = tv.rearrange("p c (t w) -> p c t w", t=8)
                # tmp main: planes p1..p7 minus h[p0..p6]; flat per comp on
                # full chunks (multi-dim APs drop DVE to 1x on HW)
                if full:
                    for c in range(2):
                        nc.vector.tensor_sub(
                            tv[:, c, W : 8 * W], xv[:, c, W : 8 * W],
                            hv[:, c, 0 : 7 * W],
                        )
                else:
                    nc.vector.tensor_sub(
                        tv4[:, :, 1:8, 0:w], xv4[:, :, 1:8, 0:w],
                        hv4[:, :, 0:7, 0:w],
                    )
                # tmp p0: minus t3 (PSUM; DVE only)
                for c in range(2):
                    nc.vector.tensor_sub(
                        tv[:, c, 0:w], xv4[:, c, 0, 0:w], xs[0][c][:, 0:w]
                    )
                # xw main (in-place): p0..p6 minus h[p1..p7]
                if full:
                    for c in range(2):
                        nc.vector.tensor_sub(
                            tv[:, c, 0 : 7 * W], tv[:, c, 0 : 7 * W],
                            hv[:, c, W : 8 * W],
                        )
                else:
                    nc.vector.tensor_sub(
                        tv4[:, :, 0:7, 0:w], tv4[:, :, 0:7, 0:w],
                        hv4[:, :, 1:8, 0:w],
                    )
                # xw p7: minus t0 (PSUM; DVE only)
                for c in range(2):
                    nc.vector.tensor_sub(
                        tv[:, c, 7 * W : 7 * W + w],
                        tv[:, c, 7 * W : 7 * W + w], xs[1][c][:, 0:w],
                    )
                return tmp

            def emit_sq_half(ci, tmp, half):
                co, w = chunks[ci]
                tv4 = tmp[:, :].rearrange("p (g t) -> p g t", g=2).rearrange(
                    "p g (t w) -> p g t w", t=8
                )[:, half : half + 1, :, 0:w]
                nc.scalar.activation(tv4, tv4, ACTF.Square)

            def emit_mel(ci, pw):
                co, w = chunks[ci]
                V = min(w - 3, NC - 3 - co)
                mp = mps.tile([128, W], f32, tag="mp")
                for ct in range(16):
                    nc.tensor.matmul(
                        mp[:, 0:V],
                        fbb[:, ct * 128 : (ct + 1) * 128],
                        pw[:, ct * W : ct * W + V],
                        start=(ct == 0),
                        stop=(ct == 15),
                    )
                nc.scalar.copy(melb[:, co : co + V], mp[:, 0:V])

            # tail scheduling: batches of b whose mel completes after chunk
            # ci; the last b is split across the final two chunks.
            ready_after = [[] for _ in range(ncb)]
            bdone = 0
            for ci, (co, w) in enumerate(chunks):
                V = min(w - 3, NC - 3 - co)
                while bdone < BL and bdone * SBLK + T <= co + V:
                    ready_after[ci].append(bdone)
                    bdone += 1

            def chain(c1, c2, mel_ap, e1_srcs=None, e1_scale=SCM / SCL):
                """PCEN pointwise chain into c2. Either e1_srcs (list of
                (psum_ap, dst_ap) for the Toeplitz smoother, scale SCM/SCL)
                or c1 pre-filled with scan state (pass e1_scale=SCM)."""
                if e1_srcs is not None:
                    for src, dst in e1_srcs:
                        nc.scalar.activation(
                            dst, src, ACTF.Ln, bias=bias_t[:, 0:1],
                            scale=e1_scale,
                        )
                else:
                    nc.scalar.activation(
                        c2, c1, ACTF.Ln, bias=bias_t[:, 0:1], scale=e1_scale
                    )
                    c1, c2 = c2, c1
                # here c1 holds e1
                nc.scalar.activation(
                    c2, c1, ACTF.Exp, bias=bias_t[:, 1:2], scale=-ALPHA
                )
                nc.vector.tensor_mul(c1, c2, mel_ap)
                nc.scalar.activation(
                    c2, c1, ACTF.Ln, bias=bias_t[:, 2:3], scale=SCM / (SCE * S)
                )
                nc.scalar.activation(
                    c1, c2, ACTF.Exp, bias=bias_t[:, 3:4], scale=R
                )
                return c1  # result

            def emit_tail(bs):
                n = len(bs)
                c1 = p_ec.tile([128, 2 * T], f32, tag="ec")
                c2 = p_ec.tile([128, 2 * T], f32, tag="ec")
                # PCEN smoother as lower-triangular Toeplitz matmul:
                # transpose melb (time onto partitions) via DMA XBAR, then
                # 3 accumulating matmuls against responsibility-masked LT.
                e1_srcs = []
                for k, b in enumerate(bs):
                    melT = p_mt.tile([128, 3 * 128], f16, tag="mt")
                    for j, (t0, lo, hi) in enumerate(LT_TILES):
                        nc.sync.dma_start_transpose(
                            melT[:, j * 128 : (j + 1) * 128],
                            melb[:, b * SBLK + t0 : b * SBLK + t0 + 128],
                        )
                    msp = mps.tile([128, W], f32, tag="mp")
                    for j in range(3):
                        nc.tensor.matmul(
                            msp[:, 0:T],
                            melT[:, j * 128 : (j + 1) * 128],
                            ltv[:, j, :],
                            start=(j == 0),
                            stop=(j == 2),
                        )
                    e1_srcs.append((msp[:, 0:T], c1[:, k * T : (k + 1) * T]))
                if n > 1:
                    mel_ap = melb[:, bs[0] * SBLK : bs[0] * SBLK + n * SBLK
                                  ].rearrange("p (k t) -> p k t", k=n)[:, :, 0:T]
                    c1a = c1[:, 0 : n * T].rearrange("p (k t) -> p k t", k=n)
                    c2a = c2[:, 0 : n * T].rearrange("p (k t) -> p k t", k=n)
                else:
                    mel_ap = melb[:, bs[0] * SBLK : bs[0] * SBLK + T]
                    c1a = c1[:, 0:T]
                    c2a = c2[:, 0:T]
                res = chain(c1a, c2a, mel_ap, e1_srcs=e1_srcs)
                # result is in c1 (e1_srcs path); final sub into the free c2
                nc.vector.tensor_scalar_sub(
                    c2[:, 0 : n * T], c1[:, 0 : n * T], DELTA**R
                )
                for k, b in enumerate(bs):
                    nc.sync.dma_start(y_d.ap()[b], c2[:, k * T : (k + 1) * T])

            sc_state = {}

            def emit_tail_part(b, t0, t1, last):
                """Split tail for the final b: scan+chain cols [t0, t1)."""
                if b not in sc_state:
                    sc_state[b] = p_sc.tile([128, T], f16, tag="sc", name="sc")
                sc = sc_state[b]
                n = t1 - t0
                nc.vector.tensor_tensor_scan(
                    sc[:, t0:t1],
                    decf[:, 0:n],
                    melb[:, b * SBLK + t0 : b * SBLK + t1],
                    0.0 if t0 == 0 else sc[:, t0 - 1 : t0],
                    ALU.mult,
                    ALU.add,
                )
                c1 = p_ec.tile([128, 2 * T], f32, tag="ec")
                c2 = p_ec.tile([128, 2 * T], f32, tag="ec")
                nc.vector.tensor_copy(c1[:, 0:n], sc[:, t0:t1])
                res = chain(
                    c1[:, 0:n], c2[:, 0:n],
                    melb[:, b * SBLK + t0 : b * SBLK + t1], e1_scale=SCM,
                )
                # scan path: result in (swapped) c2 view; sub into c1
                nc.vector.tensor_scalar_sub(c1[:, 0:n], res, DELTA**R)
                nc.sync.dma_start(y_d.ap()[b][:, t0:t1], c1[:, 0:n])

            # ---- pipelined emission ----
            # chunk 0 first (its inputs are the head DMA slices), then the
            # bulk loads, then chunk 1
            emit_u(0)
            emit_dft(0)
            emit_late_loads()
            if ncb > 1:
                emit_u(1)
                emit_dft(1)
            # the last b's tail is split across the final two chunks
            split_b = None
            if ncb >= 2 and ready_after[ncb - 1]:
                last_grp = ready_after[ncb - 1]
                split_b = last_grp[-1]
                ready_after[ncb - 1] = last_grp[:-1]
            pending_tails = []
            pending_split = None
            sc_split = 0
            for ci in range(ncb):
                if ci + 2 < ncb:
                    emit_u(ci + 2)
                x = emit_X(ci, even_on_dve=(ci == ncb - 1))
                asbs[ci] = None
                # first half of dft(ci+2) right away: at chunk boundaries the
                # PE FIFO then has ready work instead of blocking on
                # wconv-gated xs/mel
                if ci + 2 < ncb:
                    emit_dft(ci + 2, fps=range(0, 4))
                xs = emit_xs(ci, x)
                tmp = emit_wconv(ci, x, xs)
                # tails next: their chains are latency-bound and must not
                # queue behind the next chunk's evacs on the ACT FIFO
                if pending_tails:
                    emit_tail(pending_tails)
                if pending_split is not None:
                    emit_tail_part(*pending_split)
                if ci + 2 < ncb:
                    emit_dft(
                        ci + 2,
                        fps=range(4, 8),
                        mid_cb=lambda fp, i=ci, t=tmp: emit_sq_half(
                            i, t, 0 if fp == 4 else 1
                        ),
                    )
                else:
                    emit_sq_half(ci, tmp, 0)
                    emit_sq_half(ci, tmp, 1)
                emit_mel(ci, tmp)
                pending_tails = ready_after[ci]
                pending_split = None
                if split_b is not None and ci == ncb - 2:
                    co, w = chunks[ci]
                    V = min(w - 3, NC - 3 - co)
                    tcut = co + V - split_b * SBLK
                    if 0 < tcut < T:
                        pending_split = (split_b, 0, tcut, False)
                        sc_split = tcut
                    else:
                        sc_split = 0
            if pending_tails:
                emit_tail(pending_tails)
            if split_b is not None:
                emit_tail_part(split_b, sc_split, T, True)

    if split:
        _split_multiwaits(nc)
    return nc


# ---------------------------------------------------------------- host side

_CACHE = {}


def _get_consts():
    if "consts" not in _CACHE:
        E, fb2, sh, lt = _build_consts()
        # pack ft-major: per comp, per plane, nrc rc-blocks of [128, 128]
        plane_nrc = [2, 4, 4, 4, 2, 4, 4, 4]
        blocks = []
        for c in range(2):
            for p in range(8):
                nrc = plane_nrc[p]
                cols = E[: nrc * 128, c * 1024 + p * 128 : c * 1024 + (p + 1) * 128]
                for rc in range(nrc):
                    blocks.append(cols[rc * 128 : (rc + 1) * 128, :])
        # each block is [r-part 128, slot 128] with r on partitions
        e_h = np.ascontiguousarray(
            np.concatenate(blocks, axis=1).astype(np.float16)
        )
        # fb tile layout: fb_h[p, ct*128+m] = fb2[ct*128+p, m]
        fb_h = np.ascontiguousarray(
            fb2.astype(np.float16).reshape(16, 128, 128).transpose(1, 0, 2)
            .reshape(128, 2048)
        )
        sh_h = np.ascontiguousarray(sh.astype(np.float16))
        lt_h = np.ascontiguousarray(lt.astype(np.float16))
        _CACHE["consts"] = (e_h, fb_h, sh_h, lt_h)
    return _CACHE["consts"]


def _prep_core_input(wf_core):
    """wf_core: [BL, 160000] f32 -> xt [4, 128, BL*316] f16."""
    BL = wf_core.shape[0]
    x = np.pad(wf_core, ((0, 0), (PAD, PAD)), mode="reflect")
    blocks = x[:, : SBLK * HOP].reshape(BL, SBLK, HOP)
    xT = blocks.transpose(2, 0, 1).reshape(HOP, BL * SBLK)
    return np.ascontiguousarray(
        xT.astype(np.float16).reshape(4, 128, BL * SBLK)
    )


def _build_in_maps(waveform):
    e_h, fb_h, sh_h, lt_h = _get_consts()
    BL = B_TOTAL // N_CORES
    in_maps = []
    for c in range(N_CORES):
        xt = _prep_core_input(waveform[c * BL : (c + 1) * BL])
        in_maps.append(
            {"xt": xt, "e": e_h, "fb": fb_h, "sh": sh_h, "lt": lt_h}
        )
    return in_maps


def _get_nc():
    if "nc" not in _CACHE:
        _CACHE["nc"] = build_nc(BL=8)
    return _CACHE["nc"]


def kernel(waveform: np.ndarray) -> np.ndarray:
    from concourse.bass_utils import run_bass_kernel_spmd

    waveform = np.asarray(waveform, np.float32)
    assert waveform.shape == (B_TOTAL, L_WAVE)
    in_maps = _build_in_maps(waveform)
    nc = _get_nc()
    res = run_bass_kernel_spmd(nc, in_maps, core_ids=list(range(N_CORES)))
    BL = B_TOTAL // N_CORES
    out = np.empty((B_TOTAL, 1, N_MELS, T), np.float32)
    for c in range(N_CORES):
        y = np.asarray(res.results[c]["y"])  # [BL, 128, T]
        out[c * BL : (c + 1) * BL, 0] = y
    return out


# revision 67
# speedup vs baseline: 1.0035x; 1.0035x over previous
"""MelSpectrogram + PCEN Trainium2 kernel v7 (8-core data parallel).

Pipeline per core (8 batch elements):
  host: reflect-pad, hop-block transpose (512 x 2528), fp16 cast
  DVE:  u+/- = x_t +/- x_t+2; v0/v4 = u+[r'] +/- u+[r'+256]
        (radix folds: A-step and the mod-8 half-contraction)
  PE:   hop-block DFT via matmul -> A tiles (packed ft-major E, fp16,
        1/16-scaled); mod-8 planes p0/p4 contract only 256 rows against
        v0/v4; f=1024 folded into the f=0 row of the p0 tile
  ACT:  PSUM->SBUF evac (f32->f16), width-scaled
  DVE:  X-step as flat per-plane 2-dim ops (multi-dim APs and any
        concurrent gpsimd SBUF traffic both degrade DVE throughput)
  DVE:  h = 0.5 x; wconv tmp/xw as flat per-comp subs on full chunks
  PE:   q+-1 boundary shift tiles via 2 shift-matrix matmuls per comp
  ACT:  square (in-place into tmp)
  PE:   mel projection (fb folded with comp-duplication + s + scale)
  PE:   PCEN IIR smoother as Toeplitz matmuls over DMA-transposed mel
        (b0..b6); the last b uses a chained tensor_tensor_scan split
        across the final two chunks
  ACT/DVE: PCEN pointwise ln/exp chain, batched over pairs of b

Mod-8 plane-major f-slot layout per comp c (r=0: cos, i=1: -sin),
8 tiles of 128 per comp: tile p holds f = 8q+p, q=0..127; tile 0 row 0
holds f=1024. comp i tiles are offset by 8 tiles.
"""

import math
from contextlib import ExitStack

import numpy as np

SR, N_FFT, HOP, N_MELS = 32000, 2048, 512, 128
F_MIN, F_MAX = 20.0, 16000.0
EPS, S, ALPHA, DELTA, R = 1e-6, 0.025, 0.98, 2.0, 0.5
NBINS = N_FFT // 2 + 1
T = 313
SBLK = 316
PAD = N_FFT // 2
B_TOTAL, L_WAVE = 64, 160000
N_CORES = 8

SC = 16.0    # E scale (E = E_true/SC)
SCM = 16.0   # mel scale
SCE = 256.0  # e2 scale (keeps (eps+m)^-alpha comfortably in range)
SCL = 8.0    # LT scale (keeps fp16 LT entries in normal range)
W = 512
W16 = 16 * W
W8 = 8 * W

# PCEN Toeplitz tiling: three overlapping 128-frame transpose tiles;
# each LT tile only "owns" the tau rows in its responsibility range.
LT_TILES = [(0, 0, 128), (128, 128, 256), (185, 256, 313)]  # (t0, lo, hi)


def _slot_of(f, c):
    # mod-8 planes; f=1024 folded into the (unused) f=0 slot of plane 0
    if f == 1024:
        return c * 1024
    p, q = f % 8, f // 8
    return c * 1024 + p * 128 + q


def _mel_fbank():
    def hz2mel(f):
        return 2595.0 * np.log10(1.0 + np.asarray(f, np.float64) / 700.0)

    def mel2hz(m):
        return 700.0 * (10.0 ** (np.asarray(m, np.float64) / 2595.0) - 1.0)

    all_freqs = np.linspace(0.0, SR / 2.0, NBINS)
    m_pts = np.linspace(hz2mel(F_MIN), hz2mel(F_MAX), N_MELS + 2)
    f_pts = mel2hz(m_pts)
    f_diff = np.diff(f_pts)
    slopes = f_pts[None, :] - all_freqs[:, None]
    down = -slopes[:, :-2] / f_diff[:-1]
    up = slopes[:, 2:] / f_diff[1:]
    return np.maximum(0.0, np.minimum(down, up))


def _build_consts():
    r = np.arange(HOP)
    rp = np.arange(256)
    E = np.zeros((HOP, 2048), np.float64)
    for f in range(1, NBINS):  # f=0 dropped; f=1024 takes its slot
        th = 2.0 * np.pi * f * r / N_FFT
        if f % 8 in (0, 4):
            # p0/p4 contract only r' = 0..255 against v0/v4
            thp = 2.0 * np.pi * f * rp / N_FFT
            E[0:256, _slot_of(f, 0)] = np.cos(thp) / SC
            E[0:256, _slot_of(f, 1)] = -np.sin(thp) / SC
        else:
            E[:, _slot_of(f, 0)] = np.cos(th) / SC
            E[:, _slot_of(f, 1)] = -np.sin(th) / SC
    fb = _mel_fbank()
    # the slot-fold relies on fb rows 0/1/1024 being empty
    assert abs(fb[1024]).max() < 1e-9
    assert abs(fb[0]).max() < 1e-9 and abs(fb[1]).max() < 1e-9
    fb2 = np.zeros((2048, N_MELS), np.float64)
    for f in range(1024):
        wgt = fb[f] * (SC * SC / 4.0) * S / SCM
        for c in range(2):
            fb2[_slot_of(f, c)] = wgt
    # boundary-shift matrices (tmp[p0] -= 0.5 x[p7,q-1]; xw[p7] -= 0.5 x[p0,q+1])
    nsdn = -0.5 * np.eye(128, k=1)
    supl = 0.5 * np.eye(128, k=-1)
    supl[0, 127] = 0.5  # f=1023's +1 neighbor is f=1024 = p0 row 0
    sh = np.concatenate([nsdn, supl, np.eye(128)], axis=1)
    # LT[j][tau_local, t] = (1-S)^(t - tau) * SCL for tau in the tile's
    # responsibility range [lo, hi) and tau <= t (s itself is folded into
    # fb2, so melb = s*mel/SCM and msp = SCL*m/SCM).
    t = np.arange(T)
    lt = np.zeros((3, 128, T), np.float64)
    for j, (t0, lo, hi) in enumerate(LT_TILES):
        for tau in range(lo, hi):
            msk = t >= tau
            lt[j, tau - t0, msk] = (1.0 - S) ** (t[msk] - tau) * SCL
    return E, fb2, sh, lt


# Full-width chunks, then a geometrically-decreasing tail (elementwise is
# width-scaled, and the final serial drain scales with the LAST chunk).
def _make_chunks(NC):
    chunks = []
    co = 0
    while co < NC - 3:
        rem = NC - co
        if rem <= 96:
            chunks.append((co, rem)); co += rem - 3
        elif rem <= W:
            h = max(96, (rem * 3 + 4) // 5)
            chunks.append((co, h)); co += h - 3
        else:
            chunks.append((co, W)); co += W - 3
    return chunks


def _split_multiwaits(nc, limit=1):
    """This walrus build accepts at most `limit` sync-waits per instruction
    (and none at all on scalar_tensor_tensor); move excess waits onto
    preceding same-engine NoOps."""
    import bass_rust
    import concourse.mybir as mybir

    for fn in nc.m.functions:
        for b in fn.blocks:
            insts = b.instructions
            new = []
            changed = False
            for i in insts:
                lim = limit
                if (
                    isinstance(i, mybir.InstTensorScalarPtr)
                    and getattr(i, "is_scalar_tensor_tensor", False)
                    and not getattr(i, "is_tensor_tensor_scan", False)
                ):
                    lim = 0
                si = i.sync_info
                if si is not None and len(si.on_wait) > lim:
                    waits = list(si.on_wait)
                    keep = waits[len(waits) - lim :] if lim else []
                    rest = waits[: len(waits) - lim] if lim else waits
                    step = max(limit, 1)
                    for k in range(0, len(rest), step):
                        chunk = rest[k : k + step]
                        nop = mybir.InstNoOp(
                            name=f"{i.name}-wsplit{k}", ins=[], outs=[]
                        )
                        nop.engine = i.engine
                        nop.sync_info = bass_rust.SyncInfo(
                            on_wait=chunk, on_update=[]
                        )
                        new.append(nop)
                        changed = True
                    si.on_wait = keep
                new.append(i)
            if changed:
                b.instructions = new


def build_nc(BL=8, split=True):
    import concourse.bass as bass
    import concourse.mybir as mybir
    from concourse import tile

    f16 = mybir.dt.float16
    f32 = mybir.dt.float32
    ALU = mybir.AluOpType
    ACTF = mybir.ActivationFunctionType

    NC = BL * SBLK
    chunks = _make_chunks(NC)
    ncb = len(chunks)

    # packed ft-major E: per comp, planes 0..7 with 2 rc-blocks for p0/p4
    # (contraction 256 against v) and 4 for the rest -> 3584 cols per comp
    plane_nrc = [2, 4, 4, 4, 2, 4, 4, 4]
    ft_base = []
    off = 0
    for c in range(2):
        for p in range(8):
            ft_base.append(off)
            off += plane_nrc[p] * 128
    E_COLS = off  # 7168

    nc = bass.Bass("TRN2", target_bir_lowering=False, debug=False)
    xt_d = nc.dram_tensor("xt", [4, 128, NC], f16, kind="ExternalInput")
    e_d = nc.dram_tensor("e", [128, E_COLS], f16, kind="ExternalInput")
    fb_d = nc.dram_tensor("fb", [128, 2048], f16, kind="ExternalInput")
    sh_d = nc.dram_tensor("sh", [128, 384], f16, kind="ExternalInput")
    lt_d = nc.dram_tensor("lt", [3, 128, T], f16, kind="ExternalInput")
    y_d = nc.dram_tensor("y", [BL, 128, T], f32, kind="ExternalOutput")

    with tile.TileContext(nc) as tc, ExitStack() as top:
        cpool = top.enter_context(tc.tile_pool(name="consts", bufs=1))
        xb = cpool.tile([128, 4 * NC], f16)
        eb = cpool.tile([128, E_COLS], f16)
        fbb = cpool.tile([128, 2048], f16)
        melb = cpool.tile([128, NC], f16)
        shb = cpool.tile([128, 384], f16)
        ltb = cpool.tile([128, 3 * T], f16)
        decf = cpool.tile([128, T], f16)
        ltv = ltb[:, :].rearrange("p (k t) -> p k t", k=3)

        xbv = xb[:, :].rearrange("p (rc c) -> p rc c", rc=4)

        shv = shb[:, :].rearrange("p (k c) -> p k c", k=3)
        # head slices on parallel DGE queues so dft(0) can begin early;
        # everything else is issued after the first chunk's emission
        EH, XH = 768, min(516, NC)
        x_t = xt_d.ap().rearrange("rc p c -> p rc c")
        nc.sync.dma_start(xbv[:, :, 0:XH], x_t[:, :, 0:XH])
        nc.scalar.dma_start(eb[:, 0:EH], e_d.ap()[:, 0:EH])
        nc.gpsimd.dma_start(eb[:, EH : E_COLS // 2], e_d.ap()[:, EH : E_COLS // 2])
        nc.sync.dma_start(
            eb[:, E_COLS // 2 : E_COLS], e_d.ap()[:, E_COLS // 2 : E_COLS]
        )
        bias_t = cpool.tile([128, 4], f32)

        def emit_late_loads():
            nc.scalar.dma_start(shb[:, :], sh_d.ap()[:, :])
            nc.sync.dma_start(fbb[:, :], fb_d.ap()[:, :])
            if XH < NC:
                nc.scalar.dma_start(xbv[:, :, XH:NC], x_t[:, :, XH:NC])
            for k in range(3):
                nc.sync.dma_start(ltv[:, k, :], lt_d.ap()[k])
            nc.gpsimd.memset(decf[:, :], 1.0 - S)
            nc.gpsimd.memset(bias_t[:, 0:1], EPS)
            nc.gpsimd.memset(bias_t[:, 1:2], math.log(SCE))
            nc.gpsimd.memset(bias_t[:, 2:3], DELTA)
            nc.gpsimd.memset(bias_t[:, 3:4], 0.0)

        with ExitStack() as cph:
            yps = cph.enter_context(tc.tile_pool(name="yps", bufs=2, space="PSUM"))
            xsps = cph.enter_context(tc.tile_pool(name="xsps", bufs=3, space="PSUM"))
            mps = cph.enter_context(tc.tile_pool(name="mps", bufs=1, space="PSUM"))
            p_u = cph.enter_context(tc.tile_pool(name="p_u", bufs=2))
            p_v = cph.enter_context(tc.tile_pool(name="p_v", bufs=2))
            p_a = cph.enter_context(tc.tile_pool(name="p_a", bufs=3))
            p_x = cph.enter_context(tc.tile_pool(name="p_x", bufs=2))
            p_h = cph.enter_context(tc.tile_pool(name="p_h", bufs=1))
            p_t = cph.enter_context(tc.tile_pool(name="p_t", bufs=2))
            p_ec = cph.enter_context(tc.tile_pool(name="p_ec", bufs=3))
            p_sc = cph.enter_context(tc.tile_pool(name="p_sc", bufs=1))
            p_out = cph.enter_context(tc.tile_pool(name="p_out", bufs=1))

            us = [None] * ncb
            vs = [None] * ncb
            asbs = [None] * ncb

            def emit_u(cj):
                co, w = chunks[cj]
                u = p_u.tile([128, 2, 4, W], f16, tag="u")
                n2 = min(w, NC - co - 2)
                if n2 < w:
                    nc.gpsimd.memset(u[:, :, :, n2:w], 0.0)
                nc.vector.tensor_add(
                    u[:, 0, :, 0:n2], xbv[:, :, co : co + n2],
                    xbv[:, :, co + 2 : co + 2 + n2],
                )
                nc.vector.tensor_sub(
                    u[:, 1, :, 0:n2], xbv[:, :, co : co + n2],
                    xbv[:, :, co + 2 : co + 2 + n2],
                )
                v = p_v.tile([128, 2, 2, W], f16, tag="v")
                nc.vector.tensor_add(
                    v[:, 0, :, 0:w], u[:, 0, 0:2, 0:w], u[:, 0, 2:4, 0:w]
                )
                nc.vector.tensor_sub(
                    v[:, 1, :, 0:w], u[:, 0, 0:2, 0:w], u[:, 0, 2:4, 0:w]
                )
                us[cj] = u
                vs[cj] = v

            def emit_dft(cj, mid_cb=None, fps=range(8)):
                co, w = chunks[cj]
                u, v = us[cj], vs[cj]
                if asbs[cj] is None:
                    asb = p_a.tile([128, W16 + 8], f16, tag="asb")
                    asbs[cj] = asb
                    # The X-step's +1-shifted reads touch exactly column w
                    # of each of the 16 tiles (for w=W only the first pad
                    # byte). Zero them so buffer reuse never reads stale
                    # bytes.
                    if w == W:
                        nc.gpsimd.memset(asb[:, W16 : W16 + 1], 0.0)
                    else:
                        nc.gpsimd.memset(
                            asb[:, 0:W16].rearrange("p (t w) -> p t w", t=16)[
                                :, :, w : w + 1
                            ],
                            0.0,
                        )
                else:
                    asb = asbs[cj]
                for fp in fps:
                    if fp in (4, 7) and mid_cb is not None:
                        mid_cb(fp)
                    yp = yps.tile([128, 2 * W], f32, tag="yp")
                    for half in range(2):
                        ft = 2 * fp + half
                        pl = ft % 8
                        dst = yp[:, half * W : half * W + w]
                        eb0 = ft_base[ft]

                        def esl(rc):
                            return eb[:, eb0 + rc * 128 : eb0 + (rc + 1) * 128]

                        if pl in (0, 4):
                            vv = v[:, 0 if pl == 0 else 1]
                            for rc in range(2):
                                nc.tensor.matmul(
                                    dst, esl(rc), vv[:, rc, 0:w],
                                    start=(rc == 0), stop=(rc == 1),
                                )
                        else:
                            usel = 0 if pl in (2, 6) else 1
                            for rc in range(4):
                                nc.tensor.matmul(
                                    dst, esl(rc), u[:, usel, rc, 0:w],
                                    start=(rc == 0), stop=(rc == 3),
                                )
                    dst = asb[:, 2 * fp * W : (2 * fp + 2) * W].rearrange(
                        "p (t c) -> p t c", t=2
                    )[:, :, 0:w]
                    src = yp[:, :].rearrange("p (t c) -> p t c", t=2)[:, :, 0:w]
                    nc.scalar.copy(dst, src)
                asbs[cj] = asb

            def emit_X(ci, even_on_dve=False):
                co, w = chunks[ci]
                asb = asbs[ci]
                x = p_x.tile([128, W16], f16, tag="x")

                def pair(c, planes, off=0, src=None):
                    # [p, 2, 1, w] view of planes (b, b+4) of comp c
                    b = planes[0]
                    t0 = src if src is not None else asb
                    lo = c * W8 + off
                    return t0[:, lo : lo + W8].rearrange(
                        "p (a b w) -> p a b w", a=2, b=4
                    )[:, :, b : b + 1, 0:w]

                def one(c, p, off=0, src=None):
                    # flat [p, w] view of a single plane (2-dim keeps DVE 2x)
                    t0 = src if src is not None else asb
                    lo = c * W8 + p * W + off
                    return t0[:, lo : lo + w]

                # all on DVE: gpsimd shares the DVE SBUF port, and any
                # concurrent gpsimd traffic slows DVE ops ~1.8x (HW-measured)
                # even planes: aligned segmented views keep 2x; the +1-shifted
                # operand views are handled per-plane flat below
                for c in range(2):
                    for p in (0, 4):
                        nc.vector.tensor_add(
                            one(c, p, 0, x), one(c, p), one(c, p, 1)
                        )
                    for p in (2, 6):
                        nc.vector.tensor_sub(
                            one(c, p, 0, x), one(c, p), one(c, p, 1)
                        )
                # odd planes: flat per-plane DVE ops (multi-dim APs drop DVE
                # to 1x on HW, flat 2-dim stays 2x)
                for p in (1, 5):
                    # Xr = Ar + Ai[t+1]; Xi = Ai - Ar[t+1]
                    nc.vector.tensor_add(one(0, p, 0, x), one(0, p), one(1, p, 1))
                    nc.vector.tensor_sub(one(1, p, 0, x), one(1, p), one(0, p, 1))
                for p in (3, 7):
                    # Xr = Ar - Ai[t+1]; Xi = Ai + Ar[t+1]
                    nc.vector.tensor_sub(one(0, p, 0, x), one(0, p), one(1, p, 1))
                    nc.vector.tensor_add(one(1, p, 0, x), one(1, p), one(0, p, 1))
                return x

            def emit_xs(ci, x):
                co, w = chunks[ci]
                xv = x[:, :].rearrange("p (c t) -> p c t", c=2)
                t3s, t0s = [], []
                for c in range(2):
                    # t3 = x[p0] - 0.5 shift(x[p7]) = tmp[p0], evac'd by ACT
                    t3 = xsps.tile([128, W], f32, tag="xs")
                    nc.tensor.matmul(t3[:, 0:w], shv[:, 2, :],
                                     xv[:, c, 0:w],
                                     start=True, stop=False)
                    nc.tensor.matmul(t3[:, 0:w], shv[:, 0, :],
                                     xv[:, c, 7 * W : 7 * W + w],
                                     start=False, stop=True)
                    t0 = xsps.tile([128, W], f32, tag="xs")
                    nc.tensor.matmul(t0[:, 0:w], shv[:, 1, :],
                                     xv[:, c, 0:w],
                                     start=True, stop=True)
                    t3s.append(t3)
                    t0s.append(t0)
                return (t3s, t0s)

            def emit_wconv(ci, x, xs):
                co, w = chunks[ci]
                full = w == W
                xv = x[:, :].rearrange("p (c t) -> p c t", c=2)
                xv4 = xv.rearrange("p c (t w) -> p c t w", t=8)
                h = p_h.tile([128, W16], f16, tag="h")
                hv = h[:, :].rearrange("p (c t) -> p c t", c=2)
                hv4 = hv.rearrange("p c (t w) -> p c t w", t=8)
                nc.vector.tensor_scalar_mul(
                    hv4[:, :, :, 0:w], xv4[:, :, :, 0:w], 0.5
                )
                tmp = p_t.tile([128, W16], f16, tag="tmp")
                tv = tmp[:, :].rearrange("p (c t) -> p c t", c=2)
                tv4 = tv.rearrange("p c (t w) -> p c t w", t=8)
                # tmp main: planes p1..p7 minus h[p0..p6]; flat per comp on
                # full chunks (multi-dim APs drop DVE to 1x on HW)
                if full:
                    for c in range(2):
                        nc.vector.tensor_sub(
                            tv[:, c, W : 8 * W], xv[:, c, W : 8 * W],
                            hv[:, c, 0 : 7 * W],
                        )
                else:
                    nc.vector.tensor_sub(
                        tv4[:, :, 1:8, 0:w], xv4[:, :, 1:8, 0:w],
                        hv4[:, :, 0:7, 0:w],
                    )
                # tmp p0 already computed in PSUM by the xs matmuls; ACT
                # evacs it (moves boundary work off the bottleneck DVE)
                for c in range(2):
                    nc.scalar.copy(tv[:, c, 0:w], xs[0][c][:, 0:w])
                # xw main (in-place): p0..p6 minus h[p1..p7]
                if full:
                    for c in range(2):
                        nc.vector.tensor_sub(
                            tv[:, c, 0 : 7 * W], tv[:, c, 0 : 7 * W],
                            hv[:, c, W : 8 * W],
                        )
                else:
                    nc.vector.tensor_sub(
                        tv4[:, :, 0:7, 0:w], tv4[:, :, 0:7, 0:w],
                        hv4[:, :, 1:8, 0:w],
                    )
                # xw p7: minus t0 (PSUM; DVE only)
                for c in range(2):
                    nc.vector.tensor_sub(
                        tv[:, c, 7 * W : 7 * W + w],
                        tv[:, c, 7 * W : 7 * W + w], xs[1][c][:, 0:w],
                    )
                return tmp

            def emit_sq_half(ci, tmp, half):
                co, w = chunks[ci]
                tv4 = tmp[:, :].rearrange("p (g t) -> p g t", g=2).rearrange(
                    "p g (t w) -> p g t w", t=8
                )[:, half : half + 1, :, 0:w]
                nc.scalar.activation(tv4, tv4, ACTF.Square)

            def emit_mel(ci, pw):
                co, w = chunks[ci]
                V = min(w - 3, NC - 3 - co)
                mp = mps.tile([128, W], f32, tag="mp")
                for ct in range(16):
                    nc.tensor.matmul(
                        mp[:, 0:V],
                        fbb[:, ct * 128 : (ct + 1) * 128],
                        pw[:, ct * W : ct * W + V],
                        start=(ct == 0),
                        stop=(ct == 15),
                    )
                nc.scalar.copy(melb[:, co : co + V], mp[:, 0:V])

            # tail scheduling: batches of b whose mel completes after chunk
            # ci; the last b is split across the final two chunks.
            ready_after = [[] for _ in range(ncb)]
            bdone = 0
            for ci, (co, w) in enumerate(chunks):
                V = min(w - 3, NC - 3 - co)
                while bdone < BL and bdone * SBLK + T <= co + V:
                    ready_after[ci].append(bdone)
                    bdone += 1

            def chain(c1, c2, mel_ap, e1_srcs=None, e1_scale=SCM / SCL):
                """PCEN pointwise chain into c2. Either e1_srcs (list of
                (psum_ap, dst_ap) for the Toeplitz smoother, scale SCM/SCL)
                or c1 pre-filled with scan state (pass e1_scale=SCM)."""
                if e1_srcs is not None:
                    for src, dst in e1_srcs:
                        nc.scalar.activation(
                            dst, src, ACTF.Ln, bias=bias_t[:, 0:1],
                            scale=e1_scale,
                        )
                else:
                    nc.scalar.activation(
                        c2, c1, ACTF.Ln, bias=bias_t[:, 0:1], scale=e1_scale
                    )
                    c1, c2 = c2, c1
                # here c1 holds e1
                nc.scalar.activation(
                    c2, c1, ACTF.Exp, bias=bias_t[:, 1:2], scale=-ALPHA
                )
                nc.vector.tensor_mul(c1, c2, mel_ap)
                nc.scalar.activation(
                    c2, c1, ACTF.Ln, bias=bias_t[:, 2:3], scale=SCM / (SCE * S)
                )
                nc.scalar.activation(
                    c1, c2, ACTF.Exp, bias=bias_t[:, 3:4], scale=R
                )
                return c1  # result

            def emit_tail(bs):
                n = len(bs)
                c1 = p_ec.tile([128, 2 * T], f32, tag="ec")
                c2 = p_ec.tile([128, 2 * T], f32, tag="ec")
                # PCEN smoother as lower-triangular Toeplitz matmul:
                # transpose melb (time onto partitions) via DMA XBAR, then
                # 3 accumulating matmuls against responsibility-masked LT.
                e1_srcs = []
                for k, b in enumerate(bs):
                    melT = p_mt.tile([128, 3 * 128], f16, tag="mt")
                    for j, (t0, lo, hi) in enumerate(LT_TILES):
                        nc.sync.dma_start_transpose(
                            melT[:, j * 128 : (j + 1) * 128],
                            melb[:, b * SBLK + t0 : b * SBLK + t0 + 128],
                        )
                    msp = mps.tile([128, W], f32, tag="mp")
                    for j in range(3):
                        nc.tensor.matmul(
                            msp[:, 0:T],
                            melT[:, j * 128 : (j + 1) * 128],
                            ltv[:, j, :],
                            start=(j == 0),
                            stop=(j == 2),
                        )
                    e1_srcs.append((msp[:, 0:T], c1[:, k * T : (k + 1) * T]))
                if n > 1:
                    mel_ap = melb[:, bs[0] * SBLK : bs[0] * SBLK + n * SBLK
                                  ].rearrange("p (k t) -> p k t", k=n)[:, :, 0:T]
                    c1a = c1[:, 0 : n * T].rearrange("p (k t) -> p k t", k=n)
                    c2a = c2[:, 0 : n * T].rearrange("p (k t) -> p k t", k=n)
                else:
                    mel_ap = melb[:, bs[0] * SBLK : bs[0] * SBLK + T]
                    c1a = c1[:, 0:T]
                    c2a = c2[:, 0:T]
                res = chain(c1a, c2a, mel_ap, e1_srcs=e1_srcs)
                # result is in c1 (e1_srcs path); final sub into the free c2
                nc.vector.tensor_scalar_sub(
                    c2[:, 0 : n * T], c1[:, 0 : n * T], DELTA**R
                )
                for k, b in enumerate(bs):
                    nc.sync.dma_start(y_d.ap()[b], c2[:, k * T : (k + 1) * T])

            sc_state = {}

            def emit_tail_part(b, t0, t1, last):
                """Split tail for the final b: scan+chain cols [t0, t1)."""
                if b not in sc_state:
                    sc_state[b] = p_sc.tile([128, T], f16, tag="sc", name="sc")
                sc = sc_state[b]
                n = t1 - t0
                nc.vector.tensor_tensor_scan(
                    sc[:, t0:t1],
                    decf[:, 0:n],
                    melb[:, b * SBLK + t0 : b * SBLK + t1],
                    0.0 if t0 == 0 else sc[:, t0 - 1 : t0],
                    ALU.mult,
                    ALU.add,
                )
                c1 = p_ec.tile([128, 2 * T], f32, tag="ec")
                c2 = p_ec.tile([128, 2 * T], f32, tag="ec")
                nc.vector.tensor_copy(c1[:, 0:n], sc[:, t0:t1])
                res = chain(
                    c1[:, 0:n], c2[:, 0:n],
                    melb[:, b * SBLK + t0 : b * SBLK + t1], e1_scale=SCM,
                )
                # scan path: result in (swapped) c2 view; sub into c1
                nc.vector.tensor_scalar_sub(c1[:, 0:n], res, DELTA**R)
                nc.sync.dma_start(y_d.ap()[b][:, t0:t1], c1[:, 0:n])

            # ---- pipelined emission ----
            # chunk 0 first (its inputs are the head DMA slices), then the
            # bulk loads, then chunk 1
            emit_u(0)
            emit_dft(0)
            emit_late_loads()
            if ncb > 1:
                emit_u(1)
                emit_dft(1)
            # the last b's tail is split across the final two chunks
            split_b = None
            if ncb >= 2 and ready_after[ncb - 1]:
                last_grp = ready_after[ncb - 1]
                split_b = last_grp[-1]
                ready_after[ncb - 1] = last_grp[:-1]
            pending_tails = []
            pending_split = None
            sc_split = 0
            for ci in range(ncb):
                if ci + 2 < ncb:
                    emit_u(ci + 2)
                x = emit_X(ci, even_on_dve=(ci == ncb - 1))
                asbs[ci] = None
                # first half of dft(ci+2) right away: at chunk boundaries the
                # PE FIFO then has ready work instead of blocking on
                # wconv-gated xs/mel
                if ci + 2 < ncb:
                    emit_dft(ci + 2, fps=range(0, 4))
                xs = emit_xs(ci, x)
                tmp = emit_wconv(ci, x, xs)
                # tails next: their chains are latency-bound and must not
                # queue behind the next chunk's evacs on the ACT FIFO
                if pending_tails:
                    emit_tail(pending_tails)
                if pending_split is not None:
                    emit_tail_part(*pending_split)
                if ci + 2 < ncb:
                    emit_dft(
                        ci + 2,
                        fps=range(4, 8),
                        mid_cb=lambda fp, i=ci, t=tmp: emit_sq_half(
                            i, t, 0 if fp == 4 else 1
                        ),
                    )
                else:
                    emit_sq_half(ci, tmp, 0)
                    emit_sq_half(ci, tmp, 1)
                emit_mel(ci, tmp)
                pending_tails = ready_after[ci]
                pending_split = None
                if split_b is not None and ci in (ncb - 3, ncb - 2):
                    co, w = chunks[ci]
                    V = min(w - 3, NC - 3 - co)
                    tcut = min(co + V - split_b * SBLK, T)
                    if sc_split < tcut < T:
                        pending_split = (split_b, sc_split, tcut, False)
                        sc_split = tcut
            if pending_tails:
                emit_tail(pending_tails)
            if split_b is not None:
                emit_tail_part(split_b, sc_split, T, True)

    if split:
        _split_multiwaits(nc)
    return nc


# ---------------------------------------------------------------- host side

_CACHE = {}


def _get_consts():
    if "consts" not in _CACHE:
        E, fb2, sh, lt = _build_consts()
        # pack ft-major: per comp, per plane, nrc rc-blocks of [128, 128]
        plane_nrc = [2, 4, 4, 4, 2, 4, 4, 4]
        blocks = []
        for c in range(2):
            for p in range(8):
                nrc = plane_nrc[p]
                cols = E[: nrc * 128, c * 1024 + p * 128 : c * 1024 + (p + 1) * 128]
                for rc in range(nrc):
                    blocks.append(cols[rc * 128 : (rc + 1) * 128, :])
        # each block is [r-part 128, slot 128] with r on partitions
        e_h = np.ascontiguousarray(
            np.concatenate(blocks, axis=1).astype(np.float16)
        )
        # fb tile layout: fb_h[p, ct*128+m] = fb2[ct*128+p, m]
        fb_h = np.ascontiguousarray(
            fb2.astype(np.float16).reshape(16, 128, 128).transpose(1, 0, 2)
            .reshape(128, 2048)
        )
        sh_h = np.ascontiguousarray(sh.astype(np.float16))
        lt_h = np.ascontiguousarray(lt.astype(np.float16))
        _CACHE["consts"] = (e_h, fb_h, sh_h, lt_h)
    return _CACHE["consts"]


def _prep_core_input(wf_core):
    """wf_core: [BL, 160000] f32 -> xt [4, 128, BL*316] f16."""
    BL = wf_core.shape[0]
    x = np.pad(wf_core, ((0, 0), (PAD, PAD)), mode="reflect")
    blocks = x[:, : SBLK * HOP].reshape(BL, SBLK, HOP)
    xT = blocks.transpose(2, 0, 1).reshape(HOP, BL * SBLK)
    return np.ascontiguousarray(
        xT.astype(np.float16).reshape(4, 128, BL * SBLK)
    )


def _build_in_maps(waveform):
    e_h, fb_h, sh_h, lt_h = _get_consts()
    BL = B_TOTAL // N_CORES
    in_maps = []
    for c in range(N_CORES):
        xt = _prep_core_input(waveform[c * BL : (c + 1) * BL])
        in_maps.append(
            {"xt": xt, "e": e_h, "fb": fb_h, "sh": sh_h, "lt": lt_h}
        )
    return in_maps


def _get_nc():
    if "nc" not in _CACHE:
        _CACHE["nc"] = build_nc(BL=8)
    return _CACHE["nc"]


def kernel(waveform: np.ndarray) -> np.ndarray:
    from concourse.bass_utils import run_bass_kernel_spmd

    waveform = np.asarray(waveform, np.float32)
    assert waveform.shape == (B_TOTAL, L_WAVE)
    in_maps = _build_in_maps(waveform)
    nc = _get_nc()
    res = run_bass_kernel_spmd(nc, in_maps, core_ids=list(range(N_CORES)))
    BL = B_TOTAL // N_CORES
    out = np.empty((B_TOTAL, 1, N_MELS, T), np.float32)
    for c in range(N_CORES):
        y = np.asarray(res.results[c]["y"])  # [BL, 128, T]
        out[c * BL : (c + 1) * BL, 0] = y
    return out


# revision 68
# speedup vs baseline: 1.0225x; 1.0189x over previous
"""MelSpectrogram + PCEN Trainium2 kernel v7 (8-core data parallel).

Pipeline per core (8 batch elements):
  host: reflect-pad, hop-block transpose (512 x 2528), fp16 cast
  DVE:  u+/- = x_t +/- x_t+2; v0/v4 = u+[r'] +/- u+[r'+256]
        (radix folds: A-step and the mod-8 half-contraction)
  PE:   hop-block DFT via matmul -> A tiles (packed ft-major E, fp16,
        1/16-scaled); mod-8 planes p0/p4 contract only 256 rows against
        v0/v4; f=1024 folded into the f=0 row of the p0 tile
  ACT:  PSUM->SBUF evac (f32->f16), width-scaled
  DVE:  X-step as flat per-plane 2-dim ops (multi-dim APs and any
        concurrent gpsimd SBUF traffic both degrade DVE throughput)
  DVE:  h = 0.5 x; wconv tmp/xw as flat per-comp subs on full chunks
  PE:   q+-1 boundary shift tiles via 2 shift-matrix matmuls per comp
  ACT:  square (in-place into tmp)
  PE:   mel projection (fb folded with comp-duplication + s + scale)
  PE:   PCEN IIR smoother as Toeplitz matmuls over DMA-transposed mel
        (b0..b6); the last b uses a chained tensor_tensor_scan split
        across the final two chunks
  ACT/DVE: PCEN pointwise ln/exp chain, batched over pairs of b

Mod-8 plane-major f-slot layout per comp c (r=0: cos, i=1: -sin),
8 tiles of 128 per comp: tile p holds f = 8q+p, q=0..127; tile 0 row 0
holds f=1024. comp i tiles are offset by 8 tiles.
"""

import math
from contextlib import ExitStack

import numpy as np

SR, N_FFT, HOP, N_MELS = 32000, 2048, 512, 128
F_MIN, F_MAX = 20.0, 16000.0
EPS, S, ALPHA, DELTA, R = 1e-6, 0.025, 0.98, 2.0, 0.5
NBINS = N_FFT // 2 + 1
T = 313
SBLK = 316
PAD = N_FFT // 2
B_TOTAL, L_WAVE = 64, 160000
N_CORES = 8

SC = 16.0    # E scale (E = E_true/SC)
SCM = 16.0   # mel scale
SCE = 256.0  # e2 scale (keeps (eps+m)^-alpha comfortably in range)
SCL = 8.0    # LT scale (keeps fp16 LT entries in normal range)
W = 512
W16 = 16 * W
W8 = 8 * W

# PCEN Toeplitz tiling: three overlapping 128-frame transpose tiles;
# each LT tile only "owns" the tau rows in its responsibility range.
LT_TILES = [(0, 0, 128), (128, 128, 256), (185, 256, 313)]  # (t0, lo, hi)


def _slot_of(f, c):
    # mod-8 planes; f=1024 folded into the (unused) f=0 slot of plane 0
    if f == 1024:
        return c * 1024
    p, q = f % 8, f // 8
    return c * 1024 + p * 128 + q


def _mel_fbank():
    def hz2mel(f):
        return 2595.0 * np.log10(1.0 + np.asarray(f, np.float64) / 700.0)

    def mel2hz(m):
        return 700.0 * (10.0 ** (np.asarray(m, np.float64) / 2595.0) - 1.0)

    all_freqs = np.linspace(0.0, SR / 2.0, NBINS)
    m_pts = np.linspace(hz2mel(F_MIN), hz2mel(F_MAX), N_MELS + 2)
    f_pts = mel2hz(m_pts)
    f_diff = np.diff(f_pts)
    slopes = f_pts[None, :] - all_freqs[:, None]
    down = -slopes[:, :-2] / f_diff[:-1]
    up = slopes[:, 2:] / f_diff[1:]
    return np.maximum(0.0, np.minimum(down, up))


def _build_consts():
    r = np.arange(HOP)
    rp = np.arange(256)
    E = np.zeros((HOP, 2048), np.float64)
    for f in range(1, NBINS):  # f=0 dropped; f=1024 takes its slot
        th = 2.0 * np.pi * f * r / N_FFT
        if f % 8 in (0, 4):
            # p0/p4 contract only r' = 0..255 against v0/v4
            thp = 2.0 * np.pi * f * rp / N_FFT
            E[0:256, _slot_of(f, 0)] = np.cos(thp) / SC
            E[0:256, _slot_of(f, 1)] = -np.sin(thp) / SC
        else:
            E[:, _slot_of(f, 0)] = np.cos(th) / SC
            E[:, _slot_of(f, 1)] = -np.sin(th) / SC
    fb = _mel_fbank()
    # the slot-fold relies on fb rows 0/1/1024 being empty
    assert abs(fb[1024]).max() < 1e-9
    assert abs(fb[0]).max() < 1e-9 and abs(fb[1]).max() < 1e-9
    fb2 = np.zeros((2048, N_MELS), np.float64)
    for f in range(1024):
        wgt = fb[f] * (SC * SC / 4.0) * S / SCM
        for c in range(2):
            fb2[_slot_of(f, c)] = wgt
    # boundary-shift matrices (tmp[p0] -= 0.5 x[p7,q-1]; xw[p7] -= 0.5 x[p0,q+1])
    nsdn = -0.5 * np.eye(128, k=1)
    supl = 0.5 * np.eye(128, k=-1)
    supl[0, 127] = 0.5  # f=1023's +1 neighbor is f=1024 = p0 row 0
    sh = np.concatenate([nsdn, supl, np.eye(128)], axis=1)
    # LT[j][tau_local, t] = (1-S)^(t - tau) * SCL for tau in the tile's
    # responsibility range [lo, hi) and tau <= t (s itself is folded into
    # fb2, so melb = s*mel/SCM and msp = SCL*m/SCM).
    t = np.arange(T)
    lt = np.zeros((3, 128, T), np.float64)
    for j, (t0, lo, hi) in enumerate(LT_TILES):
        for tau in range(lo, hi):
            msk = t >= tau
            lt[j, tau - t0, msk] = (1.0 - S) ** (t[msk] - tau) * SCL
    return E, fb2, sh, lt


# Full-width chunks, then a geometrically-decreasing tail (elementwise is
# width-scaled, and the final serial drain scales with the LAST chunk).
def _make_chunks(NC):
    chunks = []
    co = 0
    while co < NC - 3:
        rem = NC - co
        if rem <= 96:
            chunks.append((co, rem)); co += rem - 3
        elif rem <= W:
            h = max(96, (rem * 3 + 4) // 5)
            chunks.append((co, h)); co += h - 3
        else:
            chunks.append((co, W)); co += W - 3
    return chunks


def _split_multiwaits(nc, limit=1):
    """This walrus build accepts at most `limit` sync-waits per instruction
    (and none at all on scalar_tensor_tensor); move excess waits onto
    preceding same-engine NoOps."""
    import bass_rust
    import concourse.mybir as mybir

    for fn in nc.m.functions:
        for b in fn.blocks:
            insts = b.instructions
            new = []
            changed = False
            for i in insts:
                lim = limit
                if (
                    isinstance(i, mybir.InstTensorScalarPtr)
                    and getattr(i, "is_scalar_tensor_tensor", False)
                    and not getattr(i, "is_tensor_tensor_scan", False)
                ):
                    lim = 0
                si = i.sync_info
                if si is not None and len(si.on_wait) > lim:
                    waits = list(si.on_wait)
                    keep = waits[len(waits) - lim :] if lim else []
                    rest = waits[: len(waits) - lim] if lim else waits
                    step = max(limit, 1)
                    for k in range(0, len(rest), step):
                        chunk = rest[k : k + step]
                        nop = mybir.InstNoOp(
                            name=f"{i.name}-wsplit{k}", ins=[], outs=[]
                        )
                        nop.engine = i.engine
                        nop.sync_info = bass_rust.SyncInfo(
                            on_wait=chunk, on_update=[]
                        )
                        new.append(nop)
                        changed = True
                    si.on_wait = keep
                new.append(i)
            if changed:
                b.instructions = new


def build_nc(BL=8, split=True):
    import concourse.bass as bass
    import concourse.mybir as mybir
    from concourse import tile

    f16 = mybir.dt.float16
    f32 = mybir.dt.float32
    ALU = mybir.AluOpType
    ACTF = mybir.ActivationFunctionType

    NC = BL * SBLK
    chunks = _make_chunks(NC)
    ncb = len(chunks)

    # packed ft-major E: per comp, planes 0..7 with 2 rc-blocks for p0/p4
    # (contraction 256 against v) and 4 for the rest -> 3584 cols per comp
    plane_nrc = [2, 4, 4, 4, 2, 4, 4, 4]
    ft_base = []
    off = 0
    for c in range(2):
        for p in range(8):
            ft_base.append(off)
            off += plane_nrc[p] * 128
    E_COLS = off  # 7168

    nc = bass.Bass("TRN2", target_bir_lowering=False, debug=False)
    xt_d = nc.dram_tensor("xt", [4, 128, NC], f16, kind="ExternalInput")
    e_d = nc.dram_tensor("e", [128, E_COLS], f16, kind="ExternalInput")
    fb_d = nc.dram_tensor("fb", [128, 2048], f16, kind="ExternalInput")
    sh_d = nc.dram_tensor("sh", [128, 384], f16, kind="ExternalInput")
    lt_d = nc.dram_tensor("lt", [3, 128, T], f16, kind="ExternalInput")
    y_d = nc.dram_tensor("y", [BL, 128, T], f32, kind="ExternalOutput")

    with tile.TileContext(nc) as tc, ExitStack() as top:
        cpool = top.enter_context(tc.tile_pool(name="consts", bufs=1))
        xb = cpool.tile([128, 4 * NC], f16)
        eb = cpool.tile([128, E_COLS], f16)
        fbb = cpool.tile([128, 2048], f16)
        melb = cpool.tile([128, NC], f16)
        shb = cpool.tile([128, 384], f16)
        ltb = cpool.tile([128, 3 * T], f16)
        decf = cpool.tile([128, T], f16)
        ltv = ltb[:, :].rearrange("p (k t) -> p k t", k=3)

        xbv = xb[:, :].rearrange("p (rc c) -> p rc c", rc=4)

        shv = shb[:, :].rearrange("p (k c) -> p k c", k=3)
        # head slices on parallel DGE queues so dft(0) can begin early;
        # everything else is issued after the first chunk's emission
        EH, XH = 768, min(516, NC)
        x_t = xt_d.ap().rearrange("rc p c -> p rc c")
        nc.sync.dma_start(xbv[:, :, 0:XH], x_t[:, :, 0:XH])
        nc.scalar.dma_start(eb[:, 0:EH], e_d.ap()[:, 0:EH])
        nc.gpsimd.dma_start(eb[:, EH : E_COLS // 2], e_d.ap()[:, EH : E_COLS // 2])
        nc.sync.dma_start(
            eb[:, E_COLS // 2 : E_COLS], e_d.ap()[:, E_COLS // 2 : E_COLS]
        )
        bias_t = cpool.tile([128, 4], f32)

        def emit_late_loads():
            nc.scalar.dma_start(shb[:, :], sh_d.ap()[:, :])
            nc.sync.dma_start(fbb[:, :], fb_d.ap()[:, :])
            if XH < NC:
                nc.scalar.dma_start(xbv[:, :, XH:NC], x_t[:, :, XH:NC])
            for k in range(3):
                nc.sync.dma_start(ltv[:, k, :], lt_d.ap()[k])
            nc.gpsimd.memset(decf[:, :], 1.0 - S)
            nc.gpsimd.memset(bias_t[:, 0:1], EPS)
            nc.gpsimd.memset(bias_t[:, 1:2], math.log(SCE))
            nc.gpsimd.memset(bias_t[:, 2:3], DELTA)
            nc.gpsimd.memset(bias_t[:, 3:4], 0.0)

        with ExitStack() as cph:
            yps = cph.enter_context(tc.tile_pool(name="yps", bufs=2, space="PSUM"))
            xsps = cph.enter_context(tc.tile_pool(name="xsps", bufs=3, space="PSUM"))
            mps = cph.enter_context(tc.tile_pool(name="mps", bufs=1, space="PSUM"))
            p_u = cph.enter_context(tc.tile_pool(name="p_u", bufs=2))
            p_v = cph.enter_context(tc.tile_pool(name="p_v", bufs=2))
            p_a = cph.enter_context(tc.tile_pool(name="p_a", bufs=3))
            p_x = cph.enter_context(tc.tile_pool(name="p_x", bufs=2))
            p_h = cph.enter_context(tc.tile_pool(name="p_h", bufs=1))
            p_t = cph.enter_context(tc.tile_pool(name="p_t", bufs=2))
            p_ec = cph.enter_context(tc.tile_pool(name="p_ec", bufs=3))
            p_sc = cph.enter_context(tc.tile_pool(name="p_sc", bufs=1))
            p_out = cph.enter_context(tc.tile_pool(name="p_out", bufs=1))

            us = [None] * ncb
            vs = [None] * ncb
            asbs = [None] * ncb

            def emit_u(cj):
                co, w = chunks[cj]
                u = p_u.tile([128, 2, 4, W], f16, tag="u")
                n2 = min(w, NC - co - 2)
                if n2 < w:
                    nc.gpsimd.memset(u[:, :, :, n2:w], 0.0)
                nc.vector.tensor_add(
                    u[:, 0, :, 0:n2], xbv[:, :, co : co + n2],
                    xbv[:, :, co + 2 : co + 2 + n2],
                )
                nc.vector.tensor_sub(
                    u[:, 1, :, 0:n2], xbv[:, :, co : co + n2],
                    xbv[:, :, co + 2 : co + 2 + n2],
                )
                v = p_v.tile([128, 2, 2, W], f16, tag="v")
                nc.vector.tensor_add(
                    v[:, 0, :, 0:w], u[:, 0, 0:2, 0:w], u[:, 0, 2:4, 0:w]
                )
                nc.vector.tensor_sub(
                    v[:, 1, :, 0:w], u[:, 0, 0:2, 0:w], u[:, 0, 2:4, 0:w]
                )
                us[cj] = u
                vs[cj] = v

            def emit_dft(cj, mid_cb=None, fps=range(8)):
                co, w = chunks[cj]
                u, v = us[cj], vs[cj]
                if asbs[cj] is None:
                    asb = p_a.tile([128, W16 + 8], f16, tag="asb")
                    asbs[cj] = asb
                    # The X-step's +1-shifted reads touch exactly column w
                    # of each of the 16 tiles (for w=W only the first pad
                    # byte). Zero them so buffer reuse never reads stale
                    # bytes.
                    if w == W:
                        nc.gpsimd.memset(asb[:, W16 : W16 + 1], 0.0)
                    else:
                        nc.gpsimd.memset(
                            asb[:, 0:W16].rearrange("p (t w) -> p t w", t=16)[
                                :, :, w : w + 1
                            ],
                            0.0,
                        )
                else:
                    asb = asbs[cj]
                for fp in fps:
                    if fp in (4, 7) and mid_cb is not None:
                        mid_cb(fp)
                    yp = yps.tile([128, 2 * W], f32, tag="yp")
                    for half in range(2):
                        ft = 2 * fp + half
                        pl = ft % 8
                        dst = yp[:, half * W : half * W + w]
                        eb0 = ft_base[ft]

                        def esl(rc):
                            return eb[:, eb0 + rc * 128 : eb0 + (rc + 1) * 128]

                        if pl in (0, 4):
                            vv = v[:, 0 if pl == 0 else 1]
                            for rc in range(2):
                                nc.tensor.matmul(
                                    dst, esl(rc), vv[:, rc, 0:w],
                                    start=(rc == 0), stop=(rc == 1),
                                )
                        else:
                            usel = 0 if pl in (2, 6) else 1
                            for rc in range(4):
                                nc.tensor.matmul(
                                    dst, esl(rc), u[:, usel, rc, 0:w],
                                    start=(rc == 0), stop=(rc == 3),
                                )
                    dst = asb[:, 2 * fp * W : (2 * fp + 2) * W].rearrange(
                        "p (t c) -> p t c", t=2
                    )[:, :, 0:w]
                    src = yp[:, :].rearrange("p (t c) -> p t c", t=2)[:, :, 0:w]
                    nc.scalar.copy(dst, src)
                asbs[cj] = asb

            def emit_X(ci, even_on_dve=False):
                co, w = chunks[ci]
                asb = asbs[ci]
                x = p_x.tile([128, W16], f16, tag="x")

                def pair(c, planes, off=0, src=None):
                    # [p, 2, 1, w] view of planes (b, b+4) of comp c
                    b = planes[0]
                    t0 = src if src is not None else asb
                    lo = c * W8 + off
                    return t0[:, lo : lo + W8].rearrange(
                        "p (a b w) -> p a b w", a=2, b=4
                    )[:, :, b : b + 1, 0:w]

                def one(c, p, off=0, src=None):
                    # flat [p, w] view of a single plane (2-dim keeps DVE 2x)
                    t0 = src if src is not None else asb
                    lo = c * W8 + p * W + off
                    return t0[:, lo : lo + w]

                # all on DVE: gpsimd shares the DVE SBUF port, and any
                # concurrent gpsimd traffic slows DVE ops ~1.8x (HW-measured)
                # even planes: aligned segmented views keep 2x; the +1-shifted
                # operand views are handled per-plane flat below
                for c in range(2):
                    for p in (0, 4):
                        nc.vector.tensor_add(
                            one(c, p, 0, x), one(c, p), one(c, p, 1)
                        )
                    for p in (2, 6):
                        nc.vector.tensor_sub(
                            one(c, p, 0, x), one(c, p), one(c, p, 1)
                        )
                # odd planes: flat per-plane DVE ops (multi-dim APs drop DVE
                # to 1x on HW, flat 2-dim stays 2x)
                for p in (1, 5):
                    # Xr = Ar + Ai[t+1]; Xi = Ai - Ar[t+1]
                    nc.vector.tensor_add(one(0, p, 0, x), one(0, p), one(1, p, 1))
                    nc.vector.tensor_sub(one(1, p, 0, x), one(1, p), one(0, p, 1))
                for p in (3, 7):
                    # Xr = Ar - Ai[t+1]; Xi = Ai + Ar[t+1]
                    nc.vector.tensor_sub(one(0, p, 0, x), one(0, p), one(1, p, 1))
                    nc.vector.tensor_add(one(1, p, 0, x), one(1, p), one(0, p, 1))
                return x

            def emit_xs(ci, x):
                co, w = chunks[ci]
                xv = x[:, :].rearrange("p (c t) -> p c t", c=2)
                t3s, t0s = [], []
                for c in range(2):
                    # t3 = x[p0] - 0.5 shift(x[p7]) = tmp[p0], evac'd by ACT
                    t3 = xsps.tile([128, W], f32, tag="xs")
                    nc.tensor.matmul(t3[:, 0:w], shv[:, 2, :],
                                     xv[:, c, 0:w],
                                     start=True, stop=False)
                    nc.tensor.matmul(t3[:, 0:w], shv[:, 0, :],
                                     xv[:, c, 7 * W : 7 * W + w],
                                     start=False, stop=True)
                    t0 = xsps.tile([128, W], f32, tag="xs")
                    nc.tensor.matmul(t0[:, 0:w], shv[:, 1, :],
                                     xv[:, c, 0:w],
                                     start=True, stop=True)
                    t3s.append(t3)
                    t0s.append(t0)
                return (t3s, t0s)

            def emit_wconv(ci, x, xs):
                co, w = chunks[ci]
                full = w == W
                xv = x[:, :].rearrange("p (c t) -> p c t", c=2)
                xv4 = xv.rearrange("p c (t w) -> p c t w", t=8)
                h = p_h.tile([128, W16], f16, tag="h")
                hv = h[:, :].rearrange("p (c t) -> p c t", c=2)
                hv4 = hv.rearrange("p c (t w) -> p c t w", t=8)
                nc.vector.tensor_scalar_mul(
                    hv4[:, :, :, 0:w], xv4[:, :, :, 0:w], 0.5
                )
                tmp = p_t.tile([128, W16], f16, tag="tmp")
                tv = tmp[:, :].rearrange("p (c t) -> p c t", c=2)
                tv4 = tv.rearrange("p c (t w) -> p c t w", t=8)
                # tmp main: planes p1..p7 minus h[p0..p6]; flat per comp on
                # full chunks (multi-dim APs drop DVE to 1x on HW)
                if full:
                    for c in range(2):
                        nc.vector.tensor_sub(
                            tv[:, c, W : 8 * W], xv[:, c, W : 8 * W],
                            hv[:, c, 0 : 7 * W],
                        )
                else:
                    nc.vector.tensor_sub(
                        tv4[:, :, 1:8, 0:w], xv4[:, :, 1:8, 0:w],
                        hv4[:, :, 0:7, 0:w],
                    )
                # tmp p0 already computed in PSUM by the xs matmuls; ACT
                # evacs it (moves boundary work off the bottleneck DVE)
                for c in range(2):
                    nc.scalar.copy(tv[:, c, 0:w], xs[0][c][:, 0:w])
                # xw main (in-place): p0..p6 minus h[p1..p7]
                if full:
                    for c in range(2):
                        nc.vector.tensor_sub(
                            tv[:, c, 0 : 7 * W], tv[:, c, 0 : 7 * W],
                            hv[:, c, W : 8 * W],
                        )
                else:
                    nc.vector.tensor_sub(
                        tv4[:, :, 0:7, 0:w], tv4[:, :, 0:7, 0:w],
                        hv4[:, :, 1:8, 0:w],
                    )
                # xw p7: minus t0 (PSUM; DVE only)
                for c in range(2):
                    nc.vector.tensor_sub(
                        tv[:, c, 7 * W : 7 * W + w],
                        tv[:, c, 7 * W : 7 * W + w], xs[1][c][:, 0:w],
                    )
                return tmp

            def emit_sq_half(ci, tmp, half, on_dve=False):
                co, w = chunks[ci]
                tv4 = tmp[:, :].rearrange("p (g t) -> p g t", g=2).rearrange(
                    "p g (t w) -> p g t w", t=8
                )[:, half : half + 1, :, 0:w]
                if on_dve:
                    # tail chunks: ACT is the busy engine there, DVE idles
                    nc.vector.tensor_mul(tv4, tv4, tv4)
                else:
                    nc.scalar.activation(tv4, tv4, ACTF.Square)

            def emit_mel(ci, pw):
                co, w = chunks[ci]
                V = min(w - 3, NC - 3 - co)
                mp = mps.tile([128, W], f32, tag="mp")
                for ct in range(16):
                    nc.tensor.matmul(
                        mp[:, 0:V],
                        fbb[:, ct * 128 : (ct + 1) * 128],
                        pw[:, ct * W : ct * W + V],
                        start=(ct == 0),
                        stop=(ct == 15),
                    )
                nc.scalar.copy(melb[:, co : co + V], mp[:, 0:V])

            # tail scheduling: batches of b whose mel completes after chunk
            # ci; the last b is split across the final two chunks.
            ready_after = [[] for _ in range(ncb)]
            bdone = 0
            for ci, (co, w) in enumerate(chunks):
                V = min(w - 3, NC - 3 - co)
                while bdone < BL and bdone * SBLK + T <= co + V:
                    ready_after[ci].append(bdone)
                    bdone += 1

            def chain(c1, c2, mel_ap, e1_srcs=None, e1_scale=SCM / SCL):
                """PCEN pointwise chain into c2. Either e1_srcs (list of
                (psum_ap, dst_ap) for the Toeplitz smoother, scale SCM/SCL)
                or c1 pre-filled with scan state (pass e1_scale=SCM)."""
                if e1_srcs is not None:
                    for src, dst in e1_srcs:
                        nc.scalar.activation(
                            dst, src, ACTF.Ln, bias=bias_t[:, 0:1],
                            scale=e1_scale,
                        )
                else:
                    nc.scalar.activation(
                        c2, c1, ACTF.Ln, bias=bias_t[:, 0:1], scale=e1_scale
                    )
                    c1, c2 = c2, c1
                # here c1 holds e1
                nc.scalar.activation(
                    c2, c1, ACTF.Exp, bias=bias_t[:, 1:2], scale=-ALPHA
                )
                nc.vector.tensor_mul(c1, c2, mel_ap)
                nc.scalar.activation(
                    c2, c1, ACTF.Ln, bias=bias_t[:, 2:3], scale=SCM / (SCE * S)
                )
                nc.scalar.activation(
                    c1, c2, ACTF.Exp, bias=bias_t[:, 3:4], scale=R
                )
                return c1  # result

            def emit_tail(bs):
                n = len(bs)
                c1 = p_ec.tile([128, 2 * T], f32, tag="ec")
                c2 = p_ec.tile([128, 2 * T], f32, tag="ec")
                # PCEN smoother as lower-triangular Toeplitz matmul:
                # transpose melb (time onto partitions) via DMA XBAR, then
                # 3 accumulating matmuls against responsibility-masked LT.
                e1_srcs = []
                for k, b in enumerate(bs):
                    melT = p_mt.tile([128, 3 * 128], f16, tag="mt")
                    for j, (t0, lo, hi) in enumerate(LT_TILES):
                        nc.sync.dma_start_transpose(
                            melT[:, j * 128 : (j + 1) * 128],
                            melb[:, b * SBLK + t0 : b * SBLK + t0 + 128],
                        )
                    msp = mps.tile([128, W], f32, tag="mp")
                    for j in range(3):
                        nc.tensor.matmul(
                            msp[:, 0:T],
                            melT[:, j * 128 : (j + 1) * 128],
                            ltv[:, j, :],
                            start=(j == 0),
                            stop=(j == 2),
                        )
                    e1_srcs.append((msp[:, 0:T], c1[:, k * T : (k + 1) * T]))
                if n > 1:
                    mel_ap = melb[:, bs[0] * SBLK : bs[0] * SBLK + n * SBLK
                                  ].rearrange("p (k t) -> p k t", k=n)[:, :, 0:T]
                    c1a = c1[:, 0 : n * T].rearrange("p (k t) -> p k t", k=n)
                    c2a = c2[:, 0 : n * T].rearrange("p (k t) -> p k t", k=n)
                else:
                    mel_ap = melb[:, bs[0] * SBLK : bs[0] * SBLK + T]
                    c1a = c1[:, 0:T]
                    c2a = c2[:, 0:T]
                res = chain(c1a, c2a, mel_ap, e1_srcs=e1_srcs)
                # result is in c1 (e1_srcs path); final sub into the free c2
                nc.vector.tensor_scalar_sub(
                    c2[:, 0 : n * T], c1[:, 0 : n * T], DELTA**R
                )
                for k, b in enumerate(bs):
                    nc.sync.dma_start(y_d.ap()[b], c2[:, k * T : (k + 1) * T])

            sc_state = {}

            def emit_tail_part(b, t0, t1, last):
                """Split tail for the final b: scan+chain cols [t0, t1)."""
                if b not in sc_state:
                    sc_state[b] = p_sc.tile([128, T], f16, tag="sc", name="sc")
                sc = sc_state[b]
                n = t1 - t0
                nc.vector.tensor_tensor_scan(
                    sc[:, t0:t1],
                    decf[:, 0:n],
                    melb[:, b * SBLK + t0 : b * SBLK + t1],
                    0.0 if t0 == 0 else sc[:, t0 - 1 : t0],
                    ALU.mult,
                    ALU.add,
                )
                c1 = p_ec.tile([128, 2 * T], f32, tag="ec")
                c2 = p_ec.tile([128, 2 * T], f32, tag="ec")
                nc.vector.tensor_copy(c1[:, 0:n], sc[:, t0:t1])
                res = chain(
                    c1[:, 0:n], c2[:, 0:n],
                    melb[:, b * SBLK + t0 : b * SBLK + t1], e1_scale=SCM,
                )
                # scan path: result in (swapped) c2 view; sub into c1
                nc.vector.tensor_scalar_sub(c1[:, 0:n], res, DELTA**R)
                nc.sync.dma_start(y_d.ap()[b][:, t0:t1], c1[:, 0:n])

            # ---- pipelined emission ----
            # chunk 0 first (its inputs are the head DMA slices), then the
            # bulk loads, then chunk 1
            emit_u(0)
            emit_dft(0)
            emit_late_loads()
            if ncb > 1:
                emit_u(1)
                emit_dft(1)
            # the last b's tail is split across the final two chunks
            split_b = None
            if ncb >= 2 and ready_after[ncb - 1]:
                last_grp = ready_after[ncb - 1]
                split_b = last_grp[-1]
                ready_after[ncb - 1] = last_grp[:-1]
            pending_tails = []
            pending_split = None
            sc_split = 0
            for ci in range(ncb):
                if ci + 2 < ncb:
                    emit_u(ci + 2)
                x = emit_X(ci, even_on_dve=(ci == ncb - 1))
                asbs[ci] = None
                # first half of dft(ci+2) right away: at chunk boundaries the
                # PE FIFO then has ready work instead of blocking on
                # wconv-gated xs/mel
                if ci + 2 < ncb:
                    emit_dft(ci + 2, fps=range(0, 4))
                xs = emit_xs(ci, x)
                tmp = emit_wconv(ci, x, xs)
                # tails next: their chains are latency-bound and must not
                # queue behind the next chunk's evacs on the ACT FIFO
                if pending_tails:
                    emit_tail(pending_tails)
                if pending_split is not None:
                    emit_tail_part(*pending_split)
                if ci + 2 < ncb:
                    emit_dft(
                        ci + 2,
                        fps=range(4, 8),
                        mid_cb=lambda fp, i=ci, t=tmp: emit_sq_half(
                            i, t, 0 if fp == 4 else 1
                        ),
                    )
                else:
                    emit_sq_half(ci, tmp, 0, on_dve=True)
                    emit_sq_half(ci, tmp, 1, on_dve=True)
                emit_mel(ci, tmp)
                pending_tails = ready_after[ci]
                pending_split = None
                if split_b is not None and ci in (ncb - 3, ncb - 2):
                    co, w = chunks[ci]
                    V = min(w - 3, NC - 3 - co)
                    tcut = min(co + V - split_b * SBLK, T)
                    if sc_split < tcut < T:
                        pending_split = (split_b, sc_split, tcut, False)
                        sc_split = tcut
            if pending_tails:
                emit_tail(pending_tails)
            if split_b is not None:
                emit_tail_part(split_b, sc_split, T, True)

    if split:
        _split_multiwaits(nc)
    return nc


# ---------------------------------------------------------------- host side

_CACHE = {}


def _get_consts():
    if "consts" not in _CACHE:
        E, fb2, sh, lt = _build_consts()
        # pack ft-major: per comp, per plane, nrc rc-blocks of [128, 128]
        plane_nrc = [2, 4, 4, 4, 2, 4, 4, 4]
        blocks = []
        for c in range(2):
            for p in range(8):
                nrc = plane_nrc[p]
                cols = E[: nrc * 128, c * 1024 + p * 128 : c * 1024 + (p + 1) * 128]
                for rc in range(nrc):
                    blocks.append(cols[rc * 128 : (rc + 1) * 128, :])
        # each block is [r-part 128, slot 128] with r on partitions
        e_h = np.ascontiguousarray(
            np.concatenate(blocks, axis=1).astype(np.float16)
        )
        # fb tile layout: fb_h[p, ct*128+m] = fb2[ct*128+p, m]
        fb_h = np.ascontiguousarray(
            fb2.astype(np.float16).reshape(16, 128, 128).transpose(1, 0, 2)
            .reshape(128, 2048)
        )
        sh_h = np.ascontiguousarray(sh.astype(np.float16))
        lt_h = np.ascontiguousarray(lt.astype(np.float16))
        _CACHE["consts"] = (e_h, fb_h, sh_h, lt_h)
    return _CACHE["consts"]


def _prep_core_input(wf_core):
    """wf_core: [BL, 160000] f32 -> xt [4, 128, BL*316] f16."""
    BL = wf_core.shape[0]
    x = np.pad(wf_core, ((0, 0), (PAD, PAD)), mode="reflect")
    blocks = x[:, : SBLK * HOP].reshape(BL, SBLK, HOP)
    xT = blocks.transpose(2, 0, 1).reshape(HOP, BL * SBLK)
    return np.ascontiguousarray(
        xT.astype(np.float16).reshape(4, 128, BL * SBLK)
    )


def _build_in_maps(waveform):
    e_h, fb_h, sh_h, lt_h = _get_consts()
    BL = B_TOTAL // N_CORES
    in_maps = []
    for c in range(N_CORES):
        xt = _prep_core_input(waveform[c * BL : (c + 1) * BL])
        in_maps.append(
            {"xt": xt, "e": e_h, "fb": fb_h, "sh": sh_h, "lt": lt_h}
        )
    return in_maps


def _get_nc():
    if "nc" not in _CACHE:
        _CACHE["nc"] = build_nc(BL=8)
    return _CACHE["nc"]


def kernel(waveform: np.ndarray) -> np.ndarray:
    from concourse.bass_utils import run_bass_kernel_spmd

    waveform = np.asarray(waveform, np.float32)
    assert waveform.shape == (B_TOTAL, L_WAVE)
    in_maps = _build_in_maps(waveform)
    nc = _get_nc()
    res = run_bass_kernel_spmd(nc, in_maps, core_ids=list(range(N_CORES)))
    BL = B_TOTAL // N_CORES
    out = np.empty((B_TOTAL, 1, N_MELS, T), np.float32)
    for c in range(N_CORES):
        y = np.asarray(res.results[c]["y"])  # [BL, 128, T]
        out[c * BL : (c + 1) * BL, 0] = y
    return out


# revision 69
# speedup vs baseline: 1.0234x; 1.0009x over previous
"""MelSpectrogram + PCEN Trainium2 kernel v7 (8-core data parallel).

Pipeline per core (8 batch elements):
  host: reflect-pad, hop-block transpose (512 x 2528), fp16 cast
  DVE:  u+/- = x_t +/- x_t+2; v0/v4 = u+[r'] +/- u+[r'+256]
        (radix folds: A-step and the mod-8 half-contraction)
  PE:   hop-block DFT via matmul -> A tiles (packed ft-major E, fp16,
        1/16-scaled); mod-8 planes p0/p4 contract only 256 rows against
        v0/v4; f=1024 folded into the f=0 row of the p0 tile
  ACT:  PSUM->SBUF evac (f32->f16), width-scaled
  DVE:  X-step as flat per-plane 2-dim ops (multi-dim APs and any
        concurrent gpsimd SBUF traffic both degrade DVE throughput)
  DVE:  h = 0.5 x; wconv tmp/xw as flat per-comp subs on full chunks
  PE:   q+-1 boundary shift tiles via 2 shift-matrix matmuls per comp
  ACT:  square (in-place into tmp)
  PE:   mel projection (fb folded with comp-duplication + s + scale)
  PE:   PCEN IIR smoother as Toeplitz matmuls over DMA-transposed mel
        (b0..b6); the last b uses a chained tensor_tensor_scan split
        across the final two chunks
  ACT/DVE: PCEN pointwise ln/exp chain, batched over pairs of b

Mod-8 plane-major f-slot layout per comp c (r=0: cos, i=1: -sin),
8 tiles of 128 per comp: tile p holds f = 8q+p, q=0..127; tile 0 row 0
holds f=1024. comp i tiles are offset by 8 tiles.
"""

import math
from contextlib import ExitStack

import numpy as np

SR, N_FFT, HOP, N_MELS = 32000, 2048, 512, 128
F_MIN, F_MAX = 20.0, 16000.0
EPS, S, ALPHA, DELTA, R = 1e-6, 0.025, 0.98, 2.0, 0.5
NBINS = N_FFT // 2 + 1
T = 313
SBLK = 316
PAD = N_FFT // 2
B_TOTAL, L_WAVE = 64, 160000
N_CORES = 8

SC = 16.0    # E scale (E = E_true/SC)
SCM = 16.0   # mel scale
SCE = 256.0  # e2 scale (keeps (eps+m)^-alpha comfortably in range)
SCL = 8.0    # LT scale (keeps fp16 LT entries in normal range)
W = 512
W16 = 16 * W
W8 = 8 * W

# PCEN Toeplitz tiling: three overlapping 128-frame transpose tiles;
# each LT tile only "owns" the tau rows in its responsibility range.
LT_TILES = [(0, 0, 128), (128, 128, 256), (185, 256, 313)]  # (t0, lo, hi)


def _slot_of(f, c):
    # mod-8 planes; f=1024 folded into the (unused) f=0 slot of plane 0
    if f == 1024:
        return c * 1024
    p, q = f % 8, f // 8
    return c * 1024 + p * 128 + q


def _mel_fbank():
    def hz2mel(f):
        return 2595.0 * np.log10(1.0 + np.asarray(f, np.float64) / 700.0)

    def mel2hz(m):
        return 700.0 * (10.0 ** (np.asarray(m, np.float64) / 2595.0) - 1.0)

    all_freqs = np.linspace(0.0, SR / 2.0, NBINS)
    m_pts = np.linspace(hz2mel(F_MIN), hz2mel(F_MAX), N_MELS + 2)
    f_pts = mel2hz(m_pts)
    f_diff = np.diff(f_pts)
    slopes = f_pts[None, :] - all_freqs[:, None]
    down = -slopes[:, :-2] / f_diff[:-1]
    up = slopes[:, 2:] / f_diff[1:]
    return np.maximum(0.0, np.minimum(down, up))


def _build_consts():
    r = np.arange(HOP)
    rp = np.arange(256)
    E = np.zeros((HOP, 2048), np.float64)
    for f in range(1, NBINS):  # f=0 dropped; f=1024 takes its slot
        th = 2.0 * np.pi * f * r / N_FFT
        if f % 8 in (0, 4):
            # p0/p4 contract only r' = 0..255 against v0/v4
            thp = 2.0 * np.pi * f * rp / N_FFT
            E[0:256, _slot_of(f, 0)] = np.cos(thp) / SC
            E[0:256, _slot_of(f, 1)] = -np.sin(thp) / SC
        else:
            E[:, _slot_of(f, 0)] = np.cos(th) / SC
            E[:, _slot_of(f, 1)] = -np.sin(th) / SC
    fb = _mel_fbank()
    # the slot-fold relies on fb rows 0/1/1024 being empty
    assert abs(fb[1024]).max() < 1e-9
    assert abs(fb[0]).max() < 1e-9 and abs(fb[1]).max() < 1e-9
    fb2 = np.zeros((2048, N_MELS), np.float64)
    for f in range(1024):
        wgt = fb[f] * (SC * SC / 4.0) * S / SCM
        for c in range(2):
            fb2[_slot_of(f, c)] = wgt
    # boundary-shift matrices (tmp[p0] -= 0.5 x[p7,q-1]; xw[p7] -= 0.5 x[p0,q+1])
    nsdn = -0.5 * np.eye(128, k=1)
    supl = 0.5 * np.eye(128, k=-1)
    supl[0, 127] = 0.5  # f=1023's +1 neighbor is f=1024 = p0 row 0
    sh = np.concatenate([nsdn, supl, np.eye(128)], axis=1)
    # LT[j][tau_local, t] = (1-S)^(t - tau) * SCL for tau in the tile's
    # responsibility range [lo, hi) and tau <= t (s itself is folded into
    # fb2, so melb = s*mel/SCM and msp = SCL*m/SCM).
    t = np.arange(T)
    lt = np.zeros((3, 128, T), np.float64)
    for j, (t0, lo, hi) in enumerate(LT_TILES):
        for tau in range(lo, hi):
            msk = t >= tau
            lt[j, tau - t0, msk] = (1.0 - S) ** (t[msk] - tau) * SCL
    return E, fb2, sh, lt


# Full-width chunks, then a geometrically-decreasing tail (elementwise is
# width-scaled, and the final serial drain scales with the LAST chunk).
def _make_chunks(NC):
    chunks = []
    co = 0
    while co < NC - 3:
        rem = NC - co
        if rem <= 96:
            chunks.append((co, rem)); co += rem - 3
        elif rem <= W:
            h = max(96, (rem * 3 + 4) // 5)
            chunks.append((co, h)); co += h - 3
        else:
            chunks.append((co, W)); co += W - 3
    return chunks


def _split_multiwaits(nc, limit=1):
    """This walrus build accepts at most `limit` sync-waits per instruction
    (and none at all on scalar_tensor_tensor); move excess waits onto
    preceding same-engine NoOps."""
    import bass_rust
    import concourse.mybir as mybir

    for fn in nc.m.functions:
        for b in fn.blocks:
            insts = b.instructions
            new = []
            changed = False
            for i in insts:
                lim = limit
                if (
                    isinstance(i, mybir.InstTensorScalarPtr)
                    and getattr(i, "is_scalar_tensor_tensor", False)
                    and not getattr(i, "is_tensor_tensor_scan", False)
                ):
                    lim = 0
                si = i.sync_info
                if si is not None and len(si.on_wait) > lim:
                    waits = list(si.on_wait)
                    keep = waits[len(waits) - lim :] if lim else []
                    rest = waits[: len(waits) - lim] if lim else waits
                    step = max(limit, 1)
                    for k in range(0, len(rest), step):
                        chunk = rest[k : k + step]
                        nop = mybir.InstNoOp(
                            name=f"{i.name}-wsplit{k}", ins=[], outs=[]
                        )
                        nop.engine = i.engine
                        nop.sync_info = bass_rust.SyncInfo(
                            on_wait=chunk, on_update=[]
                        )
                        new.append(nop)
                        changed = True
                    si.on_wait = keep
                new.append(i)
            if changed:
                b.instructions = new


def build_nc(BL=8, split=True):
    import concourse.bass as bass
    import concourse.mybir as mybir
    from concourse import tile

    f16 = mybir.dt.float16
    f32 = mybir.dt.float32
    ALU = mybir.AluOpType
    ACTF = mybir.ActivationFunctionType

    NC = BL * SBLK
    chunks = _make_chunks(NC)
    ncb = len(chunks)

    # packed ft-major E: per comp, planes 0..7 with 2 rc-blocks for p0/p4
    # (contraction 256 against v) and 4 for the rest -> 3584 cols per comp
    plane_nrc = [2, 4, 4, 4, 2, 4, 4, 4]
    ft_base = []
    off = 0
    for c in range(2):
        for p in range(8):
            ft_base.append(off)
            off += plane_nrc[p] * 128
    E_COLS = off  # 7168

    nc = bass.Bass("TRN2", target_bir_lowering=False, debug=False)
    xt_d = nc.dram_tensor("xt", [4, 128, NC], f16, kind="ExternalInput")
    e_d = nc.dram_tensor("e", [128, E_COLS], f16, kind="ExternalInput")
    fb_d = nc.dram_tensor("fb", [128, 2048], f16, kind="ExternalInput")
    sh_d = nc.dram_tensor("sh", [128, 384], f16, kind="ExternalInput")
    lt_d = nc.dram_tensor("lt", [3, 128, T], f16, kind="ExternalInput")
    y_d = nc.dram_tensor("y", [BL, 128, T], f32, kind="ExternalOutput")

    with tile.TileContext(nc) as tc, ExitStack() as top:
        cpool = top.enter_context(tc.tile_pool(name="consts", bufs=1))
        xb = cpool.tile([128, 4 * NC], f16)
        eb = cpool.tile([128, E_COLS], f16)
        fbb = cpool.tile([128, 2048], f16)
        melb = cpool.tile([128, NC], f16)
        shb = cpool.tile([128, 384], f16)
        ltb = cpool.tile([128, 3 * T], f16)
        decf = cpool.tile([128, T], f16)
        ltv = ltb[:, :].rearrange("p (k t) -> p k t", k=3)

        xbv = xb[:, :].rearrange("p (rc c) -> p rc c", rc=4)

        shv = shb[:, :].rearrange("p (k c) -> p k c", k=3)
        # head slices on parallel DGE queues so dft(0) can begin early;
        # everything else is issued after the first chunk's emission
        EH, XH = 768, min(516, NC)
        x_t = xt_d.ap().rearrange("rc p c -> p rc c")
        nc.sync.dma_start(xbv[:, :, 0:XH], x_t[:, :, 0:XH])
        nc.scalar.dma_start(eb[:, 0:EH], e_d.ap()[:, 0:EH])
        nc.gpsimd.dma_start(eb[:, EH : E_COLS // 2], e_d.ap()[:, EH : E_COLS // 2])
        nc.sync.dma_start(
            eb[:, E_COLS // 2 : E_COLS], e_d.ap()[:, E_COLS // 2 : E_COLS]
        )
        bias_t = cpool.tile([128, 4], f32)

        def emit_late_loads():
            nc.scalar.dma_start(shb[:, :], sh_d.ap()[:, :])
            nc.sync.dma_start(fbb[:, :], fb_d.ap()[:, :])
            if XH < NC:
                nc.scalar.dma_start(xbv[:, :, XH:NC], x_t[:, :, XH:NC])
            for k in range(3):
                nc.sync.dma_start(ltv[:, k, :], lt_d.ap()[k])
            nc.gpsimd.memset(decf[:, :], 1.0 - S)
            nc.gpsimd.memset(bias_t[:, 0:1], EPS)
            nc.gpsimd.memset(bias_t[:, 1:2], math.log(SCE))
            nc.gpsimd.memset(bias_t[:, 2:3], DELTA)
            nc.gpsimd.memset(bias_t[:, 3:4], 0.0)

        with ExitStack() as cph:
            yps = cph.enter_context(tc.tile_pool(name="yps", bufs=2, space="PSUM"))
            xsps = cph.enter_context(tc.tile_pool(name="xsps", bufs=3, space="PSUM"))
            mps = cph.enter_context(tc.tile_pool(name="mps", bufs=1, space="PSUM"))
            p_u = cph.enter_context(tc.tile_pool(name="p_u", bufs=2))
            p_v = cph.enter_context(tc.tile_pool(name="p_v", bufs=2))
            p_a = cph.enter_context(tc.tile_pool(name="p_a", bufs=3))
            p_x = cph.enter_context(tc.tile_pool(name="p_x", bufs=2))
            p_h = cph.enter_context(tc.tile_pool(name="p_h", bufs=1))
            p_t = cph.enter_context(tc.tile_pool(name="p_t", bufs=2))
            p_ec = cph.enter_context(tc.tile_pool(name="p_ec", bufs=3))
            p_sc = cph.enter_context(tc.tile_pool(name="p_sc", bufs=1))
            p_out = cph.enter_context(tc.tile_pool(name="p_out", bufs=1))

            us = [None] * ncb
            vs = [None] * ncb
            asbs = [None] * ncb

            def emit_u(cj):
                co, w = chunks[cj]
                u = p_u.tile([128, 2, 4, W], f16, tag="u")
                n2 = min(w, NC - co - 2)
                if n2 < w:
                    nc.gpsimd.memset(u[:, :, :, n2:w], 0.0)
                nc.vector.tensor_add(
                    u[:, 0, :, 0:n2], xbv[:, :, co : co + n2],
                    xbv[:, :, co + 2 : co + 2 + n2],
                )
                nc.vector.tensor_sub(
                    u[:, 1, :, 0:n2], xbv[:, :, co : co + n2],
                    xbv[:, :, co + 2 : co + 2 + n2],
                )
                v = p_v.tile([128, 2, 2, W], f16, tag="v")
                nc.vector.tensor_add(
                    v[:, 0, :, 0:w], u[:, 0, 0:2, 0:w], u[:, 0, 2:4, 0:w]
                )
                nc.vector.tensor_sub(
                    v[:, 1, :, 0:w], u[:, 0, 0:2, 0:w], u[:, 0, 2:4, 0:w]
                )
                us[cj] = u
                vs[cj] = v

            def emit_dft(cj, mid_cb=None, fps=range(8)):
                co, w = chunks[cj]
                u, v = us[cj], vs[cj]
                if asbs[cj] is None:
                    asb = p_a.tile([128, W16 + 8], f16, tag="asb")
                    asbs[cj] = asb
                    # The X-step's +1-shifted reads touch exactly column w
                    # of each of the 16 tiles (for w=W only the first pad
                    # byte). Zero them so buffer reuse never reads stale
                    # bytes.
                    if w == W:
                        nc.gpsimd.memset(asb[:, W16 : W16 + 1], 0.0)
                    else:
                        nc.gpsimd.memset(
                            asb[:, 0:W16].rearrange("p (t w) -> p t w", t=16)[
                                :, :, w : w + 1
                            ],
                            0.0,
                        )
                else:
                    asb = asbs[cj]
                for fp in fps:
                    if fp in (4, 7) and mid_cb is not None:
                        mid_cb(fp)
                    yp = yps.tile([128, 2 * W], f32, tag="yp")
                    for half in range(2):
                        ft = 2 * fp + half
                        pl = ft % 8
                        dst = yp[:, half * W : half * W + w]
                        eb0 = ft_base[ft]

                        def esl(rc):
                            return eb[:, eb0 + rc * 128 : eb0 + (rc + 1) * 128]

                        if pl in (0, 4):
                            vv = v[:, 0 if pl == 0 else 1]
                            for rc in range(2):
                                nc.tensor.matmul(
                                    dst, esl(rc), vv[:, rc, 0:w],
                                    start=(rc == 0), stop=(rc == 1),
                                )
                        else:
                            usel = 0 if pl in (2, 6) else 1
                            for rc in range(4):
                                nc.tensor.matmul(
                                    dst, esl(rc), u[:, usel, rc, 0:w],
                                    start=(rc == 0), stop=(rc == 3),
                                )
                    dst = asb[:, 2 * fp * W : (2 * fp + 2) * W].rearrange(
                        "p (t c) -> p t c", t=2
                    )[:, :, 0:w]
                    src = yp[:, :].rearrange("p (t c) -> p t c", t=2)[:, :, 0:w]
                    nc.scalar.copy(dst, src)
                asbs[cj] = asb

            def emit_X(ci, even_on_dve=False):
                co, w = chunks[ci]
                asb = asbs[ci]
                x = p_x.tile([128, W16], f16, tag="x")

                def pair(c, planes, off=0, src=None):
                    # [p, 2, 1, w] view of planes (b, b+4) of comp c
                    b = planes[0]
                    t0 = src if src is not None else asb
                    lo = c * W8 + off
                    return t0[:, lo : lo + W8].rearrange(
                        "p (a b w) -> p a b w", a=2, b=4
                    )[:, :, b : b + 1, 0:w]

                def one(c, p, off=0, src=None):
                    # flat [p, w] view of a single plane (2-dim keeps DVE 2x)
                    t0 = src if src is not None else asb
                    lo = c * W8 + p * W + off
                    return t0[:, lo : lo + w]

                # all on DVE: gpsimd shares the DVE SBUF port, and any
                # concurrent gpsimd traffic slows DVE ops ~1.8x (HW-measured)
                # even planes: aligned segmented views keep 2x; the +1-shifted
                # operand views are handled per-plane flat below
                for c in range(2):
                    for p in (0, 4):
                        nc.vector.tensor_add(
                            one(c, p, 0, x), one(c, p), one(c, p, 1)
                        )
                    for p in (2, 6):
                        nc.vector.tensor_sub(
                            one(c, p, 0, x), one(c, p), one(c, p, 1)
                        )
                # odd planes: flat per-plane DVE ops (multi-dim APs drop DVE
                # to 1x on HW, flat 2-dim stays 2x)
                for p in (1, 5):
                    # Xr = Ar + Ai[t+1]; Xi = Ai - Ar[t+1]
                    nc.vector.tensor_add(one(0, p, 0, x), one(0, p), one(1, p, 1))
                    nc.vector.tensor_sub(one(1, p, 0, x), one(1, p), one(0, p, 1))
                for p in (3, 7):
                    # Xr = Ar - Ai[t+1]; Xi = Ai + Ar[t+1]
                    nc.vector.tensor_sub(one(0, p, 0, x), one(0, p), one(1, p, 1))
                    nc.vector.tensor_add(one(1, p, 0, x), one(1, p), one(0, p, 1))
                return x

            def emit_xs(ci, x):
                co, w = chunks[ci]
                xv = x[:, :].rearrange("p (c t) -> p c t", c=2)
                t3s, t0s = [], []
                for c in range(2):
                    # t3 = x[p0] - 0.5 shift(x[p7]) = tmp[p0], evac'd by ACT
                    t3 = xsps.tile([128, W], f32, tag="xs")
                    nc.tensor.matmul(t3[:, 0:w], shv[:, 2, :],
                                     xv[:, c, 0:w],
                                     start=True, stop=False)
                    nc.tensor.matmul(t3[:, 0:w], shv[:, 0, :],
                                     xv[:, c, 7 * W : 7 * W + w],
                                     start=False, stop=True)
                    t0 = xsps.tile([128, W], f32, tag="xs")
                    nc.tensor.matmul(t0[:, 0:w], shv[:, 1, :],
                                     xv[:, c, 0:w],
                                     start=True, stop=True)
                    t3s.append(t3)
                    t0s.append(t0)
                return (t3s, t0s)

            def emit_wconv(ci, x, xs):
                co, w = chunks[ci]
                full = w == W
                xv = x[:, :].rearrange("p (c t) -> p c t", c=2)
                xv4 = xv.rearrange("p c (t w) -> p c t w", t=8)
                h = p_h.tile([128, W16], f16, tag="h")
                hv = h[:, :].rearrange("p (c t) -> p c t", c=2)
                hv4 = hv.rearrange("p c (t w) -> p c t w", t=8)
                nc.vector.tensor_scalar_mul(
                    hv4[:, :, :, 0:w], xv4[:, :, :, 0:w], 0.5
                )
                tmp = p_t.tile([128, W16], f16, tag="tmp")
                tv = tmp[:, :].rearrange("p (c t) -> p c t", c=2)
                tv4 = tv.rearrange("p c (t w) -> p c t w", t=8)
                # tmp main: planes p1..p7 minus h[p0..p6]; flat per comp on
                # full chunks (multi-dim APs drop DVE to 1x on HW)
                if full:
                    for c in range(2):
                        nc.vector.tensor_sub(
                            tv[:, c, W : 8 * W], xv[:, c, W : 8 * W],
                            hv[:, c, 0 : 7 * W],
                        )
                else:
                    nc.vector.tensor_sub(
                        tv4[:, :, 1:8, 0:w], xv4[:, :, 1:8, 0:w],
                        hv4[:, :, 0:7, 0:w],
                    )
                # tmp p0 already computed in PSUM by the xs matmuls; ACT
                # evacs it (moves boundary work off the bottleneck DVE)
                for c in range(2):
                    nc.scalar.copy(tv[:, c, 0:w], xs[0][c][:, 0:w])
                # xw main (in-place): p0..p6 minus h[p1..p7]
                if full:
                    for c in range(2):
                        nc.vector.tensor_sub(
                            tv[:, c, 0 : 7 * W], tv[:, c, 0 : 7 * W],
                            hv[:, c, W : 8 * W],
                        )
                else:
                    nc.vector.tensor_sub(
                        tv4[:, :, 0:7, 0:w], tv4[:, :, 0:7, 0:w],
                        hv4[:, :, 1:8, 0:w],
                    )
                # xw p7: minus t0 (PSUM; DVE only)
                for c in range(2):
                    nc.vector.tensor_sub(
                        tv[:, c, 7 * W : 7 * W + w],
                        tv[:, c, 7 * W : 7 * W + w], xs[1][c][:, 0:w],
                    )
                return tmp

            def emit_sq_half(ci, tmp, half, on_dve=False):
                co, w = chunks[ci]
                tv4 = tmp[:, :].rearrange("p (g t) -> p g t", g=2).rearrange(
                    "p g (t w) -> p g t w", t=8
                )[:, half : half + 1, :, 0:w]
                if on_dve:
                    # tail chunks: ACT is the busy engine there, DVE idles
                    nc.vector.tensor_mul(tv4, tv4, tv4)
                else:
                    nc.scalar.activation(tv4, tv4, ACTF.Square)

            def emit_mel(ci, pw):
                co, w = chunks[ci]
                V = min(w - 3, NC - 3 - co)
                mp = mps.tile([128, W], f32, tag="mp")
                for ct in range(16):
                    nc.tensor.matmul(
                        mp[:, 0:V],
                        fbb[:, ct * 128 : (ct + 1) * 128],
                        pw[:, ct * W : ct * W + V],
                        start=(ct == 0),
                        stop=(ct == 15),
                    )
                nc.scalar.copy(melb[:, co : co + V], mp[:, 0:V])

            # tail scheduling: batches of b whose mel completes after chunk
            # ci; the last b is split across the final two chunks.
            ready_after = [[] for _ in range(ncb)]
            bdone = 0
            for ci, (co, w) in enumerate(chunks):
                V = min(w - 3, NC - 3 - co)
                while bdone < BL and bdone * SBLK + T <= co + V:
                    ready_after[ci].append(bdone)
                    bdone += 1

            def chain(c1, c2, mel_ap, e1_srcs=None, e1_scale=SCM / SCL):
                """PCEN pointwise chain into c2. Either e1_srcs (list of
                (psum_ap, dst_ap) for the Toeplitz smoother, scale SCM/SCL)
                or c1 pre-filled with scan state (pass e1_scale=SCM)."""
                if e1_srcs is not None:
                    for src, dst in e1_srcs:
                        nc.scalar.activation(
                            dst, src, ACTF.Ln, bias=bias_t[:, 0:1],
                            scale=e1_scale,
                        )
                else:
                    nc.scalar.activation(
                        c2, c1, ACTF.Ln, bias=bias_t[:, 0:1], scale=e1_scale
                    )
                    c1, c2 = c2, c1
                # here c1 holds e1
                nc.scalar.activation(
                    c2, c1, ACTF.Exp, bias=bias_t[:, 1:2], scale=-ALPHA
                )
                nc.vector.tensor_mul(c1, c2, mel_ap)
                nc.scalar.activation(
                    c2, c1, ACTF.Ln, bias=bias_t[:, 2:3], scale=SCM / (SCE * S)
                )
                nc.scalar.activation(
                    c1, c2, ACTF.Exp, bias=bias_t[:, 3:4], scale=R
                )
                return c1  # result

            def emit_tail(bs):
                n = len(bs)
                c1 = p_ec.tile([128, 2 * T], f32, tag="ec")
                c2 = p_ec.tile([128, 2 * T], f32, tag="ec")
                # PCEN smoother as lower-triangular Toeplitz matmul:
                # transpose melb (time onto partitions) via DMA XBAR, then
                # 3 accumulating matmuls against responsibility-masked LT.
                e1_srcs = []
                for k, b in enumerate(bs):
                    melT = p_mt.tile([128, 3 * 128], f16, tag="mt")
                    for j, (t0, lo, hi) in enumerate(LT_TILES):
                        nc.sync.dma_start_transpose(
                            melT[:, j * 128 : (j + 1) * 128],
                            melb[:, b * SBLK + t0 : b * SBLK + t0 + 128],
                        )
                    msp = mps.tile([128, W], f32, tag="mp")
                    for j in range(3):
                        nc.tensor.matmul(
                            msp[:, 0:T],
                            melT[:, j * 128 : (j + 1) * 128],
                            ltv[:, j, :],
                            start=(j == 0),
                            stop=(j == 2),
                        )
                    e1_srcs.append((msp[:, 0:T], c1[:, k * T : (k + 1) * T]))
                if n > 1:
                    mel_ap = melb[:, bs[0] * SBLK : bs[0] * SBLK + n * SBLK
                                  ].rearrange("p (k t) -> p k t", k=n)[:, :, 0:T]
                    c1a = c1[:, 0 : n * T].rearrange("p (k t) -> p k t", k=n)
                    c2a = c2[:, 0 : n * T].rearrange("p (k t) -> p k t", k=n)
                else:
                    mel_ap = melb[:, bs[0] * SBLK : bs[0] * SBLK + T]
                    c1a = c1[:, 0:T]
                    c2a = c2[:, 0:T]
                res = chain(c1a, c2a, mel_ap, e1_srcs=e1_srcs)
                # result is in c1 (e1_srcs path); final sub into the free c2
                nc.vector.tensor_scalar_sub(
                    c2[:, 0 : n * T], c1[:, 0 : n * T], DELTA**R
                )
                for k, b in enumerate(bs):
                    nc.sync.dma_start(y_d.ap()[b], c2[:, k * T : (k + 1) * T])

            sc_state = {}

            def emit_tail_part(b, t0, t1, last):
                """Split tail for the final b: scan+chain cols [t0, t1)."""
                if b not in sc_state:
                    sc_state[b] = p_sc.tile([128, T], f16, tag="sc", name="sc")
                sc = sc_state[b]
                n = t1 - t0
                nc.vector.tensor_tensor_scan(
                    sc[:, t0:t1],
                    decf[:, 0:n],
                    melb[:, b * SBLK + t0 : b * SBLK + t1],
                    0.0 if t0 == 0 else sc[:, t0 - 1 : t0],
                    ALU.mult,
                    ALU.add,
                )
                c1 = p_ec.tile([128, 2 * T], f32, tag="ec")
                c2 = p_ec.tile([128, 2 * T], f32, tag="ec")
                nc.vector.tensor_copy(c1[:, 0:n], sc[:, t0:t1])
                res = chain(
                    c1[:, 0:n], c2[:, 0:n],
                    melb[:, b * SBLK + t0 : b * SBLK + t1], e1_scale=SCM,
                )
                # scan path: result in (swapped) c2 view; sub into c1
                nc.vector.tensor_scalar_sub(c1[:, 0:n], res, DELTA**R)
                nc.sync.dma_start(y_d.ap()[b][:, t0:t1], c1[:, 0:n])

            # ---- pipelined emission ----
            # PE warm-up: ~36 dummy matmuls into a never-read PSUM bank so
            # the HAM clock-gate is at 8/8 (2.4 GHz) before chunk 0's DFT;
            # cold-rate DFT otherwise gates the whole pipeline ramp. melb is
            # scratch here (its real writes come much later).
            nc.gpsimd.memset(melb[:, 0:512], 0.0)
            wps = yps.tile([128, 2 * W], f32, tag="yp")
            for k in range(36):
                nc.tensor.matmul(
                    wps[:, 0:W], melb[:, 0:128], melb[:, 0:W],
                    start=(k == 0), stop=(k == 35),
                )
            # chunk 0 first (its inputs are the head DMA slices), then the
            # bulk loads, then chunk 1
            emit_u(0)
            emit_dft(0)
            emit_late_loads()
            if ncb > 1:
                emit_u(1)
                emit_dft(1)
            # the last b's tail is split across the final two chunks
            split_b = None
            if ncb >= 2 and ready_after[ncb - 1]:
                last_grp = ready_after[ncb - 1]
                split_b = last_grp[-1]
                ready_after[ncb - 1] = last_grp[:-1]
            pending_tails = []
            pending_split = None
            sc_split = 0
            for ci in range(ncb):
                if ci + 2 < ncb:
                    emit_u(ci + 2)
                x = emit_X(ci, even_on_dve=(ci == ncb - 1))
                asbs[ci] = None
                # first half of dft(ci+2) right away: at chunk boundaries the
                # PE FIFO then has ready work instead of blocking on
                # wconv-gated xs/mel
                if ci + 2 < ncb:
                    emit_dft(ci + 2, fps=range(0, 4))
                xs = emit_xs(ci, x)
                tmp = emit_wconv(ci, x, xs)
                # tails next: their chains are latency-bound and must not
                # queue behind the next chunk's evacs on the ACT FIFO
                if pending_tails:
                    emit_tail(pending_tails)
                if pending_split is not None:
                    emit_tail_part(*pending_split)
                if ci + 2 < ncb:
                    emit_dft(
                        ci + 2,
                        fps=range(4, 8),
                        mid_cb=lambda fp, i=ci, t=tmp: emit_sq_half(
                            i, t, 0 if fp == 4 else 1
                        ),
                    )
                else:
                    emit_sq_half(ci, tmp, 0, on_dve=True)
                    emit_sq_half(ci, tmp, 1, on_dve=True)
                emit_mel(ci, tmp)
                pending_tails = ready_after[ci]
                pending_split = None
                if split_b is not None and ci in (ncb - 3, ncb - 2):
                    co, w = chunks[ci]
                    V = min(w - 3, NC - 3 - co)
                    tcut = min(co + V - split_b * SBLK, T)
                    if sc_split < tcut < T:
                        pending_split = (split_b, sc_split, tcut, False)
                        sc_split = tcut
            if pending_tails:
                emit_tail(pending_tails)
            if split_b is not None:
                emit_tail_part(split_b, sc_split, T, True)

    if split:
        _split_multiwaits(nc)
    return nc


# ---------------------------------------------------------------- host side

_CACHE = {}


def _get_consts():
    if "consts" not in _CACHE:
        E, fb2, sh, lt = _build_consts()
        # pack ft-major: per comp, per plane, nrc rc-blocks of [128, 128]
        plane_nrc = [2, 4, 4, 4, 2, 4, 4, 4]
        blocks = []
        for c in range(2):
            for p in range(8):
                nrc = plane_nrc[p]
                cols = E[: nrc * 128, c * 1024 + p * 128 : c * 1024 + (p + 1) * 128]
                for rc in range(nrc):
                    blocks.append(cols[rc * 128 : (rc + 1) * 128, :])
        # each block is [r-part 128, slot 128] with r on partitions
        e_h = np.ascontiguousarray(
            np.concatenate(blocks, axis=1).astype(np.float16)
        )
        # fb tile layout: fb_h[p, ct*128+m] = fb2[ct*128+p, m]
        fb_h = np.ascontiguousarray(
            fb2.astype(np.float16).reshape(16, 128, 128).transpose(1, 0, 2)
            .reshape(128, 2048)
        )
        sh_h = np.ascontiguousarray(sh.astype(np.float16))
        lt_h = np.ascontiguousarray(lt.astype(np.float16))
        _CACHE["consts"] = (e_h, fb_h, sh_h, lt_h)
    return _CACHE["consts"]


def _prep_core_input(wf_core):
    """wf_core: [BL, 160000] f32 -> xt [4, 128, BL*316] f16."""
    BL = wf_core.shape[0]
    x = np.pad(wf_core, ((0, 0), (PAD, PAD)), mode="reflect")
    blocks = x[:, : SBLK * HOP].reshape(BL, SBLK, HOP)
    xT = blocks.transpose(2, 0, 1).reshape(HOP, BL * SBLK)
    return np.ascontiguousarray(
        xT.astype(np.float16).reshape(4, 128, BL * SBLK)
    )


def _build_in_maps(waveform):
    e_h, fb_h, sh_h, lt_h = _get_consts()
    BL = B_TOTAL // N_CORES
    in_maps = []
    for c in range(N_CORES):
        xt = _prep_core_input(waveform[c * BL : (c + 1) * BL])
        in_maps.append(
            {"xt": xt, "e": e_h, "fb": fb_h, "sh": sh_h, "lt": lt_h}
        )
    return in_maps


def _get_nc():
    if "nc" not in _CACHE:
        _CACHE["nc"] = build_nc(BL=8)
    return _CACHE["nc"]


def kernel(waveform: np.ndarray) -> np.ndarray:
    from concourse.bass_utils import run_bass_kernel_spmd

    waveform = np.asarray(waveform, np.float32)
    assert waveform.shape == (B_TOTAL, L_WAVE)
    in_maps = _build_in_maps(waveform)
    nc = _get_nc()
    res = run_bass_kernel_spmd(nc, in_maps, core_ids=list(range(N_CORES)))
    BL = B_TOTAL // N_CORES
    out = np.empty((B_TOTAL, 1, N_MELS, T), np.float32)
    for c in range(N_CORES):
        y = np.asarray(res.results[c]["y"])  # [BL, 128, T]
        out[c * BL : (c + 1) * BL, 0] = y
    return out
